# revision 1
# baseline (speedup 1.0000x reference)
"""NT-Xent contrastive loss (forward) on 8 TRN2 NeuronCores via Bass/Tile.

Math: with h = concat(h_i, h_j) [N=8192, D=256], sim = (h @ h.T) / 0.5,
loss = mean_r( logsumexp_j(sim[r, j], j != r) - pos_r ), where
pos_r = sim[r, partner(r)] = 2 * h_i[q] . h_j[q].  The loss separates:
loss = (sum_r lse_r - sum_r pos_r) / N, and sum_r pos_r = 4 * sum(h_i * h_j).

Sharding: core c owns rows [1024c, 1024c + 1024).  Each core receives the
full transposed h, column-rotated by its row offset, so one SPMD program
serves all 8 cores: the self-similarity diagonal and the positive-pair
columns land at core-invariant positions.

Per core: the PE builds each 128-row block of sim in PSUM (bf16 operands,
fp32 accumulate) as four 1536-column chunks + two 1024-column chunks; the
diagonal is masked by accumulating I.T @ (-1e9 shifted-diag) as an extra
matmul; the scalar engine applies exp(2x - M_row) in place with a fused
row-sum (accum_out) on the 1536-chunks while the vector engine evaluates a
Schraudolph bit-trick exp (+-4%% per term, unbiased on average) on the
1024-chunks; the DVE also computes the positive-pair partial dots.  Each
core emits a [128, 52] tile of partial sums; the host finishes with
log/sum in float64.  M is a runtime input (per-row); if a row's exp-sum
under/overflows fp32, the host retries with a shifted M for those rows.
"""

import numpy as np
import ml_dtypes

B = 4096
D = 256
N = 2 * B            # 8192 rows/cols of sim
NCORES = 8
RPC = N // NCORES    # 1024 rows per core
KCH = D // 128       # 2 contraction chunks of 128
NJ = 4               # column chunks per row-block
CHUNK = N // NJ      # 2048 columns per chunk
NRB = RPC // 128     # 8 row-blocks of 128 per core
M_DEFAULT = 161.0    # logsumexp shift; safe while rowmax(2*h@h.T) in [M-70, M+79]
MASK_NEG = -1.0e9

# Schraudolph fast-exp constants (exp(y) ~= bitcast_f32(round(A*y + B)));
# B calibrated so the phase-averaged, exp-weighted relative error is ~1e-5
# (per-term max +-4%).  The DVE evaluates this for 1 of 4 column chunks per
# row-block, offloading a quarter of the exp work from the scalar engine;
# negative overflow saturates to INT_MIN = -0.0f which sums as zero.
EXP_A = float(2 ** 23 / np.log(2.0))
EXP_B = 1064865216.0

TRACE = False        # set True (e.g. from test.py) to request an NTFF trace
LAST_RESULTS = None  # BassKernelResults of the last run (for profiling)

_cache = {}


def _build():
    """Build the SPMD Bass/Tile program once per process."""
    if "nc" in _cache:
        return _cache["nc"]

    import concourse.tile as tile
    import concourse.mybir as mybir
    from concourse import bacc

    f32 = mybir.dt.float32
    bf16 = mybir.dt.bfloat16
    u32 = mybir.dt.uint32

    nc = bacc.Bacc("TRN2", target_bir_lowering=False, num_devices=NCORES)
    ht_dram = nc.dram_tensor("ht", [KCH, 128, N], bf16, kind="ExternalInput").ap()
    # eye[0][0] = I [128,128]; mask[v] [128,512] holds -1e9 at [p, 128v+p].
    # I.T @ mask[v] accumulated into a sim-block 512-slice masks its diagonal.
    eye_dram = nc.dram_tensor("eye", [1, 128, 128], bf16, kind="ExternalInput").ap()
    maskr_dram = nc.dram_tensor("maskr", [128, 4, 512], bf16, kind="ExternalInput").ap()
    bias_dram = nc.dram_tensor("biasm", [128, NRB], f32, kind="ExternalInput").ap()
    bias2_dram = nc.dram_tensor("bias2", [128, NRB], f32, kind="ExternalInput").ap()
    out_dram = nc.dram_tensor("out", [128, 52], f32, kind="ExternalOutput").ap()

    with tile.TileContext(nc) as tc:
        with (
            tc.tile_pool(name="hpool", bufs=1) as hpool,
            tc.tile_pool(name="small", bufs=1) as small,
            tc.tile_pool(name="scratch", bufs=1) as scratch,
            tc.tile_pool(name="ipool", bufs=3) as ipool,
            tc.tile_pool(name="psumA", bufs=2, space="PSUM") as psumA,
            tc.tile_pool(name="psumB", bufs=1, space="PSUM") as psumB,
        ):
            # Small constants go on the gpsimd (SWDGE) queue so they land
            # while the sync queue streams the big h.T chunks.
            eye_pos = small.tile([128, 128], bf16)
            nc.gpsimd.dma_start(out=eye_pos, in_=eye_dram[0])
            maskr_sb = small.tile([128, 4, 512], bf16)
            nc.gpsimd.dma_start(out=maskr_sb, in_=maskr_dram)
            bias_sb = small.tile([128, NRB], f32)
            nc.gpsimd.dma_start(out=bias_sb, in_=bias_dram)
            bias2_sb = small.tile([128, NRB], f32)
            nc.gpsimd.dma_start(out=bias2_sb, in_=bias2_dram)

            # Warm the ACT exp table (~2.7us load) during the DMA prologue so
            # the first real exp doesn't pay for it.
            warm_sb = small.tile([128, 1], f32)
            nc.scalar.activation(
                out=warm_sb, in_=bias_sb[:, 0:1],
                func=mybir.ActivationFunctionType.Exp, bias=0.0, scale=0.0,
            )

            # Warm the PE's HAM clock gate (cold = 1.2GHz for the first
            # ~3.4us of activity) with dummy matmuls on a memset tile while
            # the h.T DMAs are still in flight.
            wsrc = small.tile([128, 128], bf16)
            nc.vector.memset(wsrc, 0.0)
            wps = psumA.tile([128, 1536], f32, name="psA")
            for w in range(32):
                nc.tensor.matmul(
                    wps[:, (w % 3) * 512:(w % 3) * 512 + 128],
                    lhsT=wsrc, rhs=wsrc,
                    start=True, stop=True,
                )

            # h.T in SBUF on the sync HWDGE queue, in the order compute
            # consumes it.  Each DMA carries BOTH contraction halves of a
            # column range (tile layout [128, 2, width]) so the pipeline
            # never waits on a second transfer for the same columns.
            col_ranges = [(0, 1024), (1024, 2048), (2048, 3584),
                          (3584, 5120), (5120, 6656), (6656, 8192)]
            ht_tiles = []
            for di, (c0, c1) in enumerate(col_ranges):
                t = hpool.tile([128, KCH, c1 - c0], bf16, name=f"ht_{c0}")
                nc.sync.dma_start(
                    out=t,
                    in_=ht_dram[:, :, c0:c1].rearrange("k p c -> p k c"),
                )
                ht_tiles.append(t)

            def rhs_slice(k, c0, w=512):
                """[128, w] slice of rotated h.T at global column c0."""
                for (r0, r1), t in zip(col_ranges, ht_tiles):
                    if r0 <= c0 < r1:
                        assert c0 + w <= r1
                        return t[:, k, c0 - r0:c0 - r0 + w]
                raise AssertionError(c0)

            def lhsT_slice(k, rb):
                """[128, 128] row-block weights (columns rb*128..+128)."""
                return ht_tiles[0][:, k, rb * 128:(rb + 1) * 128]

            res_sb = small.tile([128, 52], f32)

            # Per row-block: 4 ACT chunks of 1536 columns (two 3-bank PSUM
            # slots) + 2 DVE fast-exp chunks of 1024 columns (one 2-bank
            # slot) = exactly 8 PSUM banks, with enough slot slack that the
            # PE never waits on a consumer.
            def emit_posdot():
                # Positive-pair partial dots: rotated columns [0,1024) are
                # this core's rows, [4096,5120) their partners.  Emitted
                # mid-stream so the DVE does it in slack, not on the tail.
                for k in range(KCH):
                    pp = scratch.tile([128, RPC], f32, name=f"ppscratch_{k}")
                    nc.vector.tensor_mul(pp, ht_tiles[0][:, k, :], ht_tiles[3][:, k, 512:512 + RPC])
                    nc.vector.reduce_sum(
                        res_sb[:, 48 + 2 * k:49 + 2 * k], pp, axis=mybir.AxisListType.X
                    )
                    nc.vector.memset(res_sb[:, 49 + 2 * k:50 + 2 * k], 0.0)

            def emit_B(rb, b):
                # DVE fast-exp chunk over columns [b*1024, b*1024+1024).
                # The diagonal (columns rb*128..+128) lies in b=0; mask it
                # with the I.T @ maskr accumulating matmul.
                psB = psumB.tile([128, 1024], f32, name="psB")
                cs0 = rb // 4
                for k in range(KCH):
                    lhsT = lhsT_slice(k, rb)
                    for cs in range(2):
                        nc.tensor.matmul(
                            psB[:, cs * 512:(cs + 1) * 512],
                            lhsT=lhsT,
                            rhs=rhs_slice(k, b * 1024 + cs * 512),
                            start=(k == 0),
                            stop=(k == KCH - 1) and not (b == 0 and cs == cs0),
                        )
                if b == 0:
                    nc.tensor.matmul(
                        psB[:, cs0 * 512:(cs0 + 1) * 512],
                        lhsT=eye_pos,
                        rhs=maskr_sb[:, rb % 4, :],
                        start=False,
                        stop=True,
                    )
                # bits = round(ps * 2A + (B - A*M_r)); sum the bitcast floats.
                ti = ipool.tile([128, 1024], u32, name="ti")
                nc.vector.tensor_scalar(
                    ti, psB, 2.0 * EXP_A, bias2_sb[:, rb:rb + 1],
                    mybir.AluOpType.mult, mybir.AluOpType.add,
                )
                nc.vector.reduce_sum(
                    res_sb[:, rb * 6 + 4 + b:rb * 6 + 5 + b],
                    ti.bitcast(f32),
                    axis=mybir.AxisListType.X,
                )

            def emit_A(rb, a):
                # ACT chunk over columns [2048 + a*1536, +1536).
                psA = psumA.tile([128, 1536], f32, name="psA")
                for k in range(KCH):
                    lhsT = lhsT_slice(k, rb)
                    for cs in range(3):
                        nc.tensor.matmul(
                            psA[:, cs * 512:(cs + 1) * 512],
                            lhsT=lhsT,
                            rhs=rhs_slice(k, 2048 + a * 1536 + cs * 512),
                            start=(k == 0),
                            stop=(k == KCH - 1),
                        )
                nc.scalar.activation(
                    out=psA,
                    in_=psA,
                    func=mybir.ActivationFunctionType.Exp,
                    bias=bias_sb[:, rb:rb + 1],
                    scale=2.0,
                    accum_out=res_sb[:, rb * 6 + a:rb * 6 + a + 1],
                )

            for rb in range(NRB):
                if rb == 5:
                    emit_posdot()
                # B (DVE) chunks interleaved between A (ACT) chunks so the
                # single B PSUM slot never stalls the PE, and each row-block
                # ends on an ACT chunk (short kernel tail).
                if rb == 0:
                    # First row-block consumes columns strictly in DMA
                    # arrival order.
                    for c in (("B", 0), ("B", 1), ("A", 0), ("A", 1),
                              ("A", 2), ("A", 3)):
                        (emit_B if c[0] == "B" else emit_A)(rb, c[1])
                else:
                    emit_B(rb, 0)
                    emit_A(rb, 0)
                    emit_B(rb, 1)
                    emit_A(rb, 1)
                    emit_A(rb, 2)
                    emit_A(rb, 3)

            # Ship rb0-6 partials while rb7 is still computing; only a
            # tiny transfer remains on the kernel tail.
            nc.sync.dma_start(out=out_dram[:, 0:42], in_=res_sb[:, 0:42])
            nc.sync.dma_start(out=out_dram[:, 42:52], in_=res_sb[:, 42:52])

    nc.compile()
    _cache["nc"] = nc
    return nc


def _make_static_inputs(h_i, h_j):
    """Per-core rotated h.T (bf16) plus the diag mask (shared)."""
    h = np.concatenate([np.asarray(h_i), np.asarray(h_j)], axis=0).astype(np.float32)
    hT = np.ascontiguousarray(h.T)  # [256, 8192]
    hts = []
    for c in range(NCORES):
        htc = np.roll(hT, -RPC * c, axis=1)
        hts.append(
            np.ascontiguousarray(htc.astype(ml_dtypes.bfloat16).reshape(KCH, 128, N))
        )
    eye = np.zeros((1, 128, 128), dtype=ml_dtypes.bfloat16)
    p = np.arange(128)
    eye[0, p, p] = 1.0
    maskr = np.zeros((128, 4, 512), dtype=ml_dtypes.bfloat16)
    for v in range(4):
        maskr[p, v, 128 * v + p] = MASK_NEG
    return hts, eye, maskr


def _axon_reset():
    """Recover the axon-tunneled NeuronCores if a previous process left them
    in an unrecoverable state."""
    try:
        import ctypes

        lib = ctypes.CDLL("/opt/axon/libaxon_pjrt.so")
        lib.axon_reset.restype = ctypes.c_int64
        return lib.axon_reset() == 0
    except Exception:
        return False


def _run(nc, hts, eye, maskr, M_per_core):
    global LAST_RESULTS
    from concourse import bass_utils

    in_maps = [
        {
            "ht": hts[c],
            "eye": eye,
            "maskr": maskr,
            "biasm": (-M_per_core[c]).astype(np.float32),
            "bias2": (EXP_B - EXP_A * M_per_core[c]).astype(np.float32),
        }
        for c in range(NCORES)
    ]
    try:
        results = bass_utils.run_bass_kernel_spmd(
            nc, in_maps, core_ids=list(range(NCORES)), trace=TRACE
        )
    except Exception:
        # A wedged accelerator (e.g. NRT_EXEC_UNIT_UNRECOVERABLE from an
        # earlier crashed process) survives process restarts; reset and retry.
        if not _axon_reset():
            raise
        results = bass_utils.run_bass_kernel_spmd(
            nc, in_maps, core_ids=list(range(NCORES)), trace=TRACE
        )
    LAST_RESULTS = results
    return results.results


def kernel(h_i, h_j):
    nc = _build()
    hts, eye, maskr = _make_static_inputs(h_i, h_j)

    # Per-core, per-row logsumexp shift M (as the activation bias -M).
    M = [np.full((128, NRB), M_DEFAULT, dtype=np.float64) for _ in range(NCORES)]

    lse = [np.full((128, NRB), np.nan) for _ in range(NCORES)]
    total_pd = 0.0

    for attempt in range(4):
        res = _run(nc, hts, eye, maskr, M)
        any_bad = False
        for c in range(NCORES):
            out = res[c]["out"].astype(np.float64)
            S = out[:, :48].reshape(128, NRB, 6).sum(axis=2)
            if attempt == 0:
                total_pd += out[:, 48:52].sum()
            good = np.isfinite(S) & (S > 0.0)
            upd = good & ~np.isfinite(lse[c])
            lse[c][upd] = M[c][upd] + np.log(S[upd])
            bad = ~np.isfinite(lse[c])
            if bad.any():
                any_bad = True
                # S == 0 -> M too high for those rows; S inf/nan -> too low.
                over = bad & ~np.isfinite(S)
                under = bad & ~over
                M[c][under] -= 75.0
                M[c][over] += 75.0
        if not any_bad:
            break

    total_lse = sum(l.sum() for l in lse)
    loss = (total_lse - 2.0 * total_pd) / float(N)
    return np.array(loss, dtype=np.float32)


if __name__ == "__main__":
    # Smoke test with random data (not the reference inputs).
    rng = np.random.default_rng(0)
    h_i = rng.standard_normal((B, D), dtype=np.float32)
    h_j = rng.standard_normal((B, D), dtype=np.float32)
    print("loss:", kernel(h_i, h_j))



# revision 5
# speedup vs baseline: 1.2594x; 1.2594x over previous
"""NT-Xent contrastive loss (forward) on 8 TRN2 NeuronCores via Bass/Tile.

Math: with h = concat(h_i, h_j) [N=8192, D=256], sim = (h @ h.T) / 0.5,
loss = mean_r( logsumexp_j(sim[r, j], j != r) - pos_r ), pos_r = 2 h_i[q].h_j[q].

sim is symmetric, so with a uniform logsumexp shift M the exp'd matrix
E = exp(sim - M) is symmetric too and each unordered block pair is computed
once: a circulant schedule where global row-block R (128 rows) covers the 33
column-blocks at cyclic offsets 0..32.  Offsets 1..31 contribute their row
sums to R's rows (free-axis accumulate) and their column sums (ones.T @ E on
the PE) to the transposed rows; offset 0 is the diagonal block (self-sim
masked by accumulating I.T @ (-1e9 diag) into PSUM); offset 32 is computed
by both endpoints, row-sums only.  Core c owns global row-blocks 8c..8c+7
with all columns pre-rotated by 1024c on the host, so one SPMD program
serves all 8 cores.

The sim blocks are produced by fp8(e4m3) DoubleRow matmuls (K=256 in one
pass, fp32 PSUM accumulate).  Per stripe of 4224 columns: two 1536-column
chunks go to the scalar engine (exact exp, bf16 out, fused row-sum) and
two 512 + one 128 column chunks go to the vector engine as a Schraudolph
u16 bit-trick exp whose bit pattern IS the bf16 value (round(A16*y+B16)),
so the same tile feeds both the row-reduce and the column-sum matmul.
The host assembles S_r from row/column partial sums, takes log in float64,
and computes the positive-pair term directly from the fp32 inputs.
"""

import numpy as np
import ml_dtypes

B = 4096
D = 256
N = 2 * B              # 8192 rows/cols of sim
NCORES = 8
NSTRIPE = 8            # row-blocks (stripes) per core
W = 4224               # stripe width: 33 blocks of 128 (cyclic offsets 0..32)
M_DEFAULT = 161.0      # logsumexp shift; safe while rowmax(2*h@h.T) in [M-70, M+79]
MASK_NEG = -1.0e9

# Schraudolph fast-exp in bf16 bit space: exp(y) ~= bitcast_bf16(round(A16*y+B16)).
A16 = 128.0 / np.log(2.0)
B16 = 16256.0 - 7.446   # offset calibrated to zero the exp-weighted mean error

# ht DMA pieces (local column ranges); every 512-aligned-128-offset chunk AP
# below fits inside exactly one piece (piece = min(x//1024, 4)).
PIECES = [(0, 1536), (1024, 2560), (2048, 3584), (3072, 4608), (4096, 5120)]
HTW = sum(e - s for s, e in PIECES)   # 7168

# per-stripe chunk layout (offsets relative to stripe start 128j)
#   A1 [0,1536) ACT   D1 [1536,2048) DVE   A2 [2048,3584) ACT
#   D2 [3584,4096) DVE   O3 [4096,4224) DVE (offset-32 block, both sides)
# colsum strips (psum row -> relative column range, all within one E tile):
CS_STRIPS = [(0, 128, 640), (1, 640, 1152), (2, 1536, 2048), (3, 2048, 2560),
             (4, 2560, 3072), (5, 3072, 3584), (6, 3584, 4096), (7, 1152, 1536)]

TRACE = False
LAST_RESULTS = None

_cache = {}


def _build():
    if "nc" in _cache:
        return _cache["nc"]

    import concourse.tile as tile
    import concourse.mybir as mybir
    from concourse import bacc

    f32 = mybir.dt.float32
    bf16 = mybir.dt.bfloat16
    fp8 = mybir.dt.float8e4
    u16 = mybir.dt.uint16
    DR = mybir.MatmulPerfMode.DoubleRow

    nc = bacc.Bacc("TRN2", target_bir_lowering=False, num_devices=NCORES)
    ht_dram = nc.dram_tensor("ht", [2, 128, HTW], fp8, kind="ExternalInput").ap()
    eye_dram = nc.dram_tensor("eye", [128, 128], bf16, kind="ExternalInput").ap()
    mskd_dram = nc.dram_tensor("mskd", [128, 128], bf16, kind="ExternalInput").ap()
    sel_dram = nc.dram_tensor("sel", [128, 8, 8], bf16, kind="ExternalInput").ap()
    rs_dram = nc.dram_tensor("rs", [128, 5 * NSTRIPE], f32, kind="ExternalOutput").ap()
    cs_dram = nc.dram_tensor("cs", [NSTRIPE, 8, 512], f32, kind="ExternalOutput").ap()

    TS1 = 2.0 * A16                  # psum holds X = a.a ; bits = A16*(2X - M) + B16
    TS2 = B16 - A16 * M_DEFAULT

    with tile.TileContext(nc) as tc:
        with (
            tc.tile_pool(name="hp", bufs=1) as hp,
            tc.tile_pool(name="small", bufs=1) as small,
            tc.tile_pool(name="ep", bufs=2) as ep,
            tc.tile_pool(name="psA", bufs=2, space="PSUM") as psA,
            tc.tile_pool(name="psB", bufs=1, space="PSUM") as psB,
            tc.tile_pool(name="psCS", bufs=1, space="PSUM") as psCS,
        ):
            # small constants on the SWDGE queue while sync streams ht
            eye_sb = small.tile([128, 128], bf16)
            nc.gpsimd.dma_start(out=eye_sb, in_=eye_dram)
            mskd_sb = small.tile([128, 128], bf16)
            nc.gpsimd.dma_start(out=mskd_sb, in_=mskd_dram)
            sel_sb = small.tile([128, 8, 8], bf16)
            nc.gpsimd.dma_start(out=sel_sb, in_=sel_dram)

            # per-partition ACT bias (-M) and exp-table warm tile
            biasm_sb = small.tile([128, 1], f32)
            nc.vector.memset(biasm_sb, -M_DEFAULT)
            warm_sb = small.tile([128, 1], f32)
            nc.vector.memset(warm_sb, 0.0)
            nc.scalar.activation(
                out=warm_sb, in_=warm_sb,
                func=mybir.ActivationFunctionType.Exp, bias=0.0, scale=0.0,
            )

            # warm the PE HAM clock gate with dummy matmuls while DMAs land
            wsrc = small.tile([128, 128], bf16)
            nc.vector.memset(wsrc, 0.0)
            wps = psA.tile([128, 1536], f32, name="psA")
            for w in range(28):
                nc.tensor.matmul(
                    wps[:, (w % 3) * 512:(w % 3) * 512 + 128],
                    lhsT=wsrc, rhs=wsrc, start=True, stop=True,
                )

            # ht pieces in consumption order on the sync HWDGE queue
            ht_tiles = []
            for (c0, c1) in PIECES:
                t = hp.tile([128, 2, c1 - c0], fp8, name=f"ht_{c0}")
                off = sum(e - s for s, e in PIECES[:len(ht_tiles)])
                nc.sync.dma_start(
                    out=t,
                    in_=ht_dram[:, :, off:off + (c1 - c0)].rearrange("k p c -> p k c"),
                )
                ht_tiles.append(t)

            def rhs8(x, wdt=512):
                """[128, 2, wdt] fp8 slice of rotated ht at local column x."""
                p = min(x // 1024, 4)
                s, e = PIECES[p]
                assert s <= x and x + wdt <= e, (x, wdt)
                return ht_tiles[p][:, :, x - s:x - s + wdt]

            res_sb = small.tile([128, 5 * NSTRIPE], f32)

            for j in range(NSTRIPE):
                base = 128 * j
                lhsT = rhs8(base, 128)          # [128, 2, 128] stripe weights
                rcol = 5 * j

                # ---- produce sim chunks (PSUM, X = a.a units) ----
                pA1 = psA.tile([128, 1536], f32, name="psA")
                for cs in range(3):
                    nc.tensor.matmul(
                        pA1[:, cs * 512:(cs + 1) * 512],
                        lhsT=lhsT, rhs=rhs8(base + cs * 512),
                        start=True, stop=(cs != 0), perf_mode=DR,
                    )
                # mask self-similarity: += mskd[m, n] over cols [0,128)
                nc.tensor.matmul(
                    pA1[:, 0:128], lhsT=eye_sb, rhs=mskd_sb,
                    start=False, stop=True,
                )
                pD1 = psB.tile([128, 512], f32, name="psB")
                nc.tensor.matmul(pD1, lhsT=lhsT, rhs=rhs8(base + 1536),
                                 start=True, stop=True, perf_mode=DR)
                pA2 = psA.tile([128, 1536], f32, name="psA")
                for cs in range(3):
                    nc.tensor.matmul(
                        pA2[:, cs * 512:(cs + 1) * 512],
                        lhsT=lhsT, rhs=rhs8(base + 2048 + cs * 512),
                        start=True, stop=True, perf_mode=DR,
                    )
                pD2 = psB.tile([128, 512], f32, name="psB")
                nc.tensor.matmul(pD2, lhsT=lhsT, rhs=rhs8(base + 3584),
                                 start=True, stop=True, perf_mode=DR)
                pO3 = psB.tile([128, 128], f32, name="psB")
                nc.tensor.matmul(pO3, lhsT=lhsT, rhs=rhs8(base + 4096, 128),
                                 start=True, stop=True, perf_mode=DR)

                # ---- exp + row sums ----
                eA1 = ep.tile([128, 1536], bf16, name="eA1")
                nc.scalar.activation(
                    out=eA1, in_=pA1, func=mybir.ActivationFunctionType.Exp,
                    bias=biasm_sb, scale=2.0,
                    accum_out=res_sb[:, rcol:rcol + 1],
                )
                tD1 = ep.tile([128, 512], u16, name="tD1")
                nc.vector.tensor_scalar(
                    tD1, pD1, TS1, TS2, mybir.AluOpType.mult, mybir.AluOpType.add)
                nc.vector.reduce_sum(
                    res_sb[:, rcol + 1:rcol + 2], tD1.bitcast(bf16),
                    axis=mybir.AxisListType.X)
                eA2 = ep.tile([128, 1536], bf16, name="eA2")
                nc.scalar.activation(
                    out=eA2, in_=pA2, func=mybir.ActivationFunctionType.Exp,
                    bias=biasm_sb, scale=2.0,
                    accum_out=res_sb[:, rcol + 2:rcol + 3],
                )
                tD2 = ep.tile([128, 512], u16, name="tD2")
                nc.vector.tensor_scalar(
                    tD2, pD2, TS1, TS2, mybir.AluOpType.mult, mybir.AluOpType.add)
                nc.vector.reduce_sum(
                    res_sb[:, rcol + 3:rcol + 4], tD2.bitcast(bf16),
                    axis=mybir.AxisListType.X)
                tO3 = ep.tile([128, 128], u16, name="tO3")
                nc.vector.tensor_scalar(
                    tO3, pO3, TS1, TS2, mybir.AluOpType.mult, mybir.AluOpType.add)
                nc.vector.reduce_sum(
                    res_sb[:, rcol + 4:rcol + 5], tO3.bitcast(bf16),
                    axis=mybir.AxisListType.X)

                # ---- column sums: cstile[i, :] = ones.T @ E_strip_i ----
                def e_slice(lo, hi):
                    if hi <= 1536:
                        return eA1[:, lo:hi]
                    if lo >= 1536 and hi <= 2048:
                        return tD1.bitcast(bf16)[:, lo - 1536:hi - 1536]
                    if lo >= 2048 and hi <= 3584:
                        return eA2[:, lo - 2048:hi - 2048]
                    return tD2.bitcast(bf16)[:, lo - 3584:hi - 3584]

                cstile = psCS.tile([8, 512], f32, name="psCS")
                for k, (row, lo, hi) in enumerate(CS_STRIPS):
                    nc.tensor.matmul(
                        cstile[:, 0:hi - lo],
                        lhsT=sel_sb[:, row, :], rhs=e_slice(lo, hi),
                        start=(k == 0), stop=(k == len(CS_STRIPS) - 1),
                    )
                csb = ep.tile([8, 512], f32, name="csb")
                nc.vector.tensor_copy(csb, cstile)
                nc.sync.dma_start(out=cs_dram[j], in_=csb)

                if j == NSTRIPE - 2:
                    nc.sync.dma_start(out=rs_dram[:, 0:5 * (NSTRIPE - 1)],
                                      in_=res_sb[:, 0:5 * (NSTRIPE - 1)])
            nc.sync.dma_start(out=rs_dram[:, 5 * (NSTRIPE - 1):],
                              in_=res_sb[:, 5 * (NSTRIPE - 1):])

    nc.compile()
    _cache["nc"] = nc
    return nc


def _make_static_inputs(h_i, h_j):
    h = np.concatenate([np.asarray(h_i), np.asarray(h_j)], axis=0).astype(np.float32)
    hT = np.ascontiguousarray(h.T)  # [256, 8192]
    hts = []
    for c in range(NCORES):
        htc = np.roll(hT, -B // 4 * c, axis=1)  # rotate by 1024*c
        pieces = [htc[:, s:e] for s, e in PIECES]
        cat = np.concatenate(pieces, axis=1)            # [256, HTW]
        hts.append(np.ascontiguousarray(
            cat.astype(ml_dtypes.float8_e4m3).reshape(2, 128, HTW)))
    p = np.arange(128)
    eye = np.zeros((128, 128), dtype=ml_dtypes.bfloat16)
    eye[p, p] = 1.0
    mskd = np.zeros((128, 128), dtype=ml_dtypes.bfloat16)
    mskd[p, p] = MASK_NEG
    sel = np.zeros((128, 8, 8), dtype=ml_dtypes.bfloat16)
    for i in range(8):
        sel[:, i, i] = 1.0
    return hts, eye, mskd, sel


def _assembly_indices():
    """Global-column index map for the colsum strips: [core, stripe, row, 512]."""
    idx = np.zeros((NCORES, NSTRIPE, 8, 512), dtype=np.int64)
    valid = np.zeros((NCORES, NSTRIPE, 8, 512), dtype=np.float64)
    for c in range(NCORES):
        for j in range(NSTRIPE):
            for row, lo, hi in CS_STRIPS:
                w = hi - lo
                loc = 128 * j + lo + np.arange(w)
                idx[c, j, row, :w] = (loc + 1024 * c) % N
                valid[c, j, row, :w] = 1.0
    return idx, valid


_IDX, _VALID = _assembly_indices()


def _axon_reset():
    try:
        import ctypes
        lib = ctypes.CDLL("/opt/axon/libaxon_pjrt.so")
        lib.axon_reset.restype = ctypes.c_int64
        return lib.axon_reset() == 0
    except Exception:
        return False


def _run(nc, hts, eye, mskd, sel):
    global LAST_RESULTS
    from concourse import bass_utils

    in_maps = [
        {"ht": hts[c], "eye": eye, "mskd": mskd, "sel": sel}
        for c in range(NCORES)
    ]
    try:
        results = bass_utils.run_bass_kernel_spmd(
            nc, in_maps, core_ids=list(range(NCORES)), trace=TRACE
        )
    except Exception:
        if not _axon_reset():
            raise
        results = bass_utils.run_bass_kernel_spmd(
            nc, in_maps, core_ids=list(range(NCORES)), trace=TRACE
        )
    LAST_RESULTS = results
    return results.results


def _host_fallback(h_i, h_j):
    """Exact float64 loss on the host (used only if the device result is
    numerically out of range for the fixed logsumexp shift)."""
    h = np.concatenate([np.asarray(h_i), np.asarray(h_j)], 0).astype(np.float64)
    sim = 2.0 * (h @ h.T)
    np.fill_diagonal(sim, -np.inf)
    m = sim.max(1)
    lse = m + np.log(np.exp(sim - m[:, None]).sum(1))
    pos = 2.0 * (h[:B] * h[B:]).sum(1)
    return np.float32((lse - np.concatenate([pos, pos])).mean())


def kernel(h_i, h_j):
    nc = _build()
    hts, eye, mskd, sel = _make_static_inputs(h_i, h_j)
    res = _run(nc, hts, eye, mskd, sel)

    S = np.zeros(N, dtype=np.float64)
    for c in range(NCORES):
        rs = res[c]["rs"].astype(np.float64)            # [128, 40]
        cs = res[c]["cs"].astype(np.float64)            # [8, 8, 512]
        # row sums: stripe j covers global rows 1024c + 128j + p
        rows = (1024 * c + (128 * np.arange(NSTRIPE))[:, None]
                + np.arange(128)[None, :])              # [8, 128]
        S[rows.ravel()] += rs.reshape(128, NSTRIPE, 5).sum(2).T.ravel()
        # column sums
        S += np.bincount(_IDX[c].ravel(),
                         weights=(cs * _VALID[c]).ravel(), minlength=N)

    if not (np.isfinite(S).all() and (S > 0.0).all()):
        return _host_fallback(h_i, h_j)

    lse = M_DEFAULT + np.log(S)
    h_i64 = np.asarray(h_i, dtype=np.float64)
    h_j64 = np.asarray(h_j, dtype=np.float64)
    pos = 2.0 * (h_i64 * h_j64).sum(1)
    loss = lse.mean() - pos.mean()
    return np.array(loss, dtype=np.float32)


if __name__ == "__main__":
    rng = np.random.default_rng(0)
    h_i = rng.standard_normal((B, D), dtype=np.float32)
    h_j = rng.standard_normal((B, D), dtype=np.float32)
    print("loss:", kernel(h_i, h_j))


# revision 9
# speedup vs baseline: 1.2677x; 1.0066x over previous
"""NT-Xent contrastive loss (forward) on 8 TRN2 NeuronCores via Bass/Tile.

Math: with h = concat(h_i, h_j) [N=8192, D=256], sim = (h @ h.T) / 0.5,
loss = mean_r( logsumexp_j(sim[r, j], j != r) - pos_r ), pos_r = 2 h_i[q].h_j[q].

sim is symmetric, so with a uniform logsumexp shift M the exp'd matrix
E = exp(sim - M) is symmetric too and each unordered block pair is computed
once: a circulant schedule where global row-block R (128 rows) covers the 33
column-blocks at cyclic offsets 0..32.  Offsets 1..31 contribute their row
sums to R's rows (free-axis accumulate) and their column sums (ones.T @ E on
the PE) to the transposed rows; offset 0 is the diagonal block (self-sim
masked by accumulating I.T @ (-1e9 diag) into PSUM); offset 32 is computed
by both endpoints, row-sums only.  Core c owns global row-blocks 8c..8c+7
with all columns pre-rotated by 1024c on the host, so one SPMD program
serves all 8 cores.

The sim blocks are produced by fp8(e4m3) DoubleRow matmuls (K=256 in one
pass, fp32 PSUM accumulate).  Per stripe of 4224 columns: two 1536-column
chunks go to the scalar engine (exact exp, bf16 out, fused row-sum) and
two 512 + one 128 column chunks go to the vector engine as a Schraudolph
u16 bit-trick exp whose bit pattern IS the bf16 value (round(A16*y+B16)),
so the same tile feeds both the row-reduce and the column-sum matmul.
The host assembles S_r from row/column partial sums, takes log in float64,
and computes the positive-pair term directly from the fp32 inputs.
"""

import numpy as np
import ml_dtypes

B = 4096
D = 256
N = 2 * B              # 8192 rows/cols of sim
NCORES = 8
NSTRIPE = 8            # row-blocks (stripes) per core
W = 4224               # stripe width: 33 blocks of 128 (cyclic offsets 0..32)
M_DEFAULT = 161.0      # logsumexp shift; safe while rowmax(2*h@h.T) in [M-70, M+79]
MASK_NEG = -1.0e9

# Schraudolph fast-exp in bf16 bit space: exp(y) ~= bitcast_bf16(round(A16*y+B16)).
A16 = 128.0 / np.log(2.0)
B16 = 16256.0 - 7.446   # offset calibrated to zero the exp-weighted mean error

# ht DMA pieces (local column ranges); every 512-aligned-128-offset chunk AP
# below fits inside exactly one piece (piece = min(x//1024, 4)).
PIECES = [(0, 1536), (1024, 2560), (2048, 3584), (3072, 4608), (4096, 5120)]
HTW = sum(e - s for s, e in PIECES)   # 7168

# per-stripe chunk layout (offsets relative to stripe start 128j)
#   A1 [0,1536) ACT   D1 [1536,2048) DVE   A2 [2048,3584) ACT
#   D2 [3584,4096) DVE   O3 [4096,4224) DVE (offset-32 block, both sides)
# The three DVE chunks share one u16 bits tile tD[0:512|512:1024|1024:1152]
# reduced by a single 1152-wide row-sum.
# colsum strips (psum row -> relative column range, all within one E tile);
# row 0 must be 512 wide (it carries start=True for the whole psum tile).
CS_STRIPS = [(0, 128, 640), (1, 640, 1152), (2, 1152, 1536), (3, 1536, 2048),
             (4, 3584, 4096), (5, 2048, 2560), (6, 2560, 3072), (7, 3072, 3584)]

TRACE = False
LAST_RESULTS = None

_cache = {}


def _build():
    if "nc" in _cache:
        return _cache["nc"]

    import concourse.tile as tile
    import concourse.mybir as mybir
    from concourse import bacc

    f32 = mybir.dt.float32
    bf16 = mybir.dt.bfloat16
    fp8 = mybir.dt.float8e4
    u16 = mybir.dt.uint16
    DR = mybir.MatmulPerfMode.DoubleRow

    nc = bacc.Bacc("TRN2", target_bir_lowering=False, num_devices=NCORES)
    ht_dram = nc.dram_tensor("ht", [2, 128, HTW], fp8, kind="ExternalInput").ap()
    eye_dram = nc.dram_tensor("eye", [128, 128], bf16, kind="ExternalInput").ap()
    mskd_dram = nc.dram_tensor("mskd", [128, 128], bf16, kind="ExternalInput").ap()
    sel_dram = nc.dram_tensor("sel", [128, 8, 8], bf16, kind="ExternalInput").ap()
    rs_dram = nc.dram_tensor("rs", [128, 3 * NSTRIPE], f32, kind="ExternalOutput").ap()
    cs_dram = nc.dram_tensor("cs", [NSTRIPE, 8, 512], f32, kind="ExternalOutput").ap()

    TS1 = 2.0 * A16                  # psum holds X = a.a ; bits = A16*(2X - M) + B16
    TS2 = B16 - A16 * M_DEFAULT

    with tile.TileContext(nc) as tc:
        with (
            tc.tile_pool(name="hp", bufs=1) as hp,
            tc.tile_pool(name="small", bufs=1) as small,
            tc.tile_pool(name="ep", bufs=2) as ep,
            tc.tile_pool(name="psA", bufs=2, space="PSUM") as psA,
            tc.tile_pool(name="psB", bufs=1, space="PSUM") as psB,
            tc.tile_pool(name="psCS", bufs=1, space="PSUM") as psCS,
        ):
            # small constants on the SWDGE queue while sync streams ht
            eye_sb = small.tile([128, 128], bf16)
            nc.gpsimd.dma_start(out=eye_sb, in_=eye_dram)
            mskd_sb = small.tile([128, 128], bf16)
            nc.gpsimd.dma_start(out=mskd_sb, in_=mskd_dram)
            sel_sb = small.tile([128, 8, 8], bf16)
            nc.gpsimd.dma_start(out=sel_sb, in_=sel_dram)

            # per-partition ACT bias (-M) and exp-table warm tile
            biasm_sb = small.tile([128, 1], f32)
            nc.vector.memset(biasm_sb, -M_DEFAULT)
            warm_sb = small.tile([128, 1], f32)
            nc.vector.memset(warm_sb, 0.0)
            nc.scalar.activation(
                out=warm_sb, in_=warm_sb,
                func=mybir.ActivationFunctionType.Exp, bias=0.0, scale=0.0,
            )

            # warm the PE HAM clock gate with dummy matmuls while DMAs land
            wsrc = small.tile([128, 128], bf16)
            nc.vector.memset(wsrc, 0.0)
            wps = psA.tile([128, 1536], f32, name="psA")
            for w in range(28):
                nc.tensor.matmul(
                    wps[:, (w % 3) * 512:(w % 3) * 512 + 128],
                    lhsT=wsrc, rhs=wsrc, start=True, stop=True,
                )

            # ht pieces in consumption order on the sync HWDGE queue
            ht_tiles = []
            for (c0, c1) in PIECES:
                t = hp.tile([128, 2, c1 - c0], fp8, name=f"ht_{c0}")
                off = sum(e - s for s, e in PIECES[:len(ht_tiles)])
                nc.sync.dma_start(
                    out=t,
                    in_=ht_dram[:, :, off:off + (c1 - c0)].rearrange("k p c -> p k c"),
                )
                ht_tiles.append(t)

            def rhs8(x, wdt=512):
                """[128, 2, wdt] fp8 slice of rotated ht at local column x."""
                p = min(x // 1024, 4)
                s, e = PIECES[p]
                assert s <= x and x + wdt <= e, (x, wdt)
                return ht_tiles[p][:, :, x - s:x - s + wdt]

            res_sb = small.tile([128, 3 * NSTRIPE], f32)

            def emit_stripe(j):
                """Produce + exp + row sums for stripe j; returns E tiles."""
                base = 128 * j
                lhsT = rhs8(base, 128)          # [128, 2, 128] stripe weights
                rcol = 3 * j

                pA1 = psA.tile([128, 1536], f32, name="psA")
                for cs in range(3):
                    nc.tensor.matmul(
                        pA1[:, cs * 512:(cs + 1) * 512],
                        lhsT=lhsT, rhs=rhs8(base + cs * 512),
                        start=True, stop=(cs != 0), perf_mode=DR,
                    )
                # mask self-similarity: += mskd[m, n] over cols [0,128)
                nc.tensor.matmul(
                    pA1[:, 0:128], lhsT=eye_sb, rhs=mskd_sb,
                    start=False, stop=True,
                )
                eA1 = ep.tile([128, 1536], bf16, name="eA1")
                nc.scalar.activation(
                    out=eA1, in_=pA1, func=mybir.ActivationFunctionType.Exp,
                    bias=biasm_sb, scale=2.0,
                    accum_out=res_sb[:, rcol:rcol + 1],
                )

                tD = ep.tile([128, 1152], u16, name="tD")
                pD1 = psB.tile([128, 512], f32, name="psB")
                nc.tensor.matmul(pD1, lhsT=lhsT, rhs=rhs8(base + 1536),
                                 start=True, stop=True, perf_mode=DR)
                nc.vector.tensor_scalar(
                    tD[:, 0:512], pD1, TS1, TS2,
                    mybir.AluOpType.mult, mybir.AluOpType.add)

                pA2 = psA.tile([128, 1536], f32, name="psA")
                for cs in range(3):
                    nc.tensor.matmul(
                        pA2[:, cs * 512:(cs + 1) * 512],
                        lhsT=lhsT, rhs=rhs8(base + 2048 + cs * 512),
                        start=True, stop=True, perf_mode=DR,
                    )
                eA2 = ep.tile([128, 1536], bf16, name="eA2")
                nc.scalar.activation(
                    out=eA2, in_=pA2, func=mybir.ActivationFunctionType.Exp,
                    bias=biasm_sb, scale=2.0,
                    accum_out=res_sb[:, rcol + 1:rcol + 2],
                )

                pD2 = psB.tile([128, 512], f32, name="psB")
                nc.tensor.matmul(pD2, lhsT=lhsT, rhs=rhs8(base + 3584),
                                 start=True, stop=True, perf_mode=DR)
                nc.vector.tensor_scalar(
                    tD[:, 512:1024], pD2, TS1, TS2,
                    mybir.AluOpType.mult, mybir.AluOpType.add)
                pO3 = psB.tile([128, 128], f32, name="psB")
                nc.tensor.matmul(pO3, lhsT=lhsT, rhs=rhs8(base + 4096, 128),
                                 start=True, stop=True, perf_mode=DR)
                nc.vector.tensor_scalar(
                    tD[:, 1024:1152], pO3, TS1, TS2,
                    mybir.AluOpType.mult, mybir.AluOpType.add)
                nc.vector.reduce_sum(
                    res_sb[:, rcol + 2:rcol + 3], tD.bitcast(bf16),
                    axis=mybir.AxisListType.X)
                return eA1, eA2, tD

            def emit_cs(j, tiles):
                """Column sums of stripe j (one stripe behind the produce)."""
                eA1, eA2, tD = tiles

                def e_slice(lo, hi):
                    if hi <= 1536:
                        return eA1[:, lo:hi]
                    if lo >= 1536 and hi <= 2048:
                        return tD.bitcast(bf16)[:, lo - 1536:hi - 1536]
                    if lo >= 2048 and hi <= 3584:
                        return eA2[:, lo - 2048:hi - 2048]
                    return tD.bitcast(bf16)[:, lo - 3584 + 512:hi - 3584 + 512]

                cstile = psCS.tile([8, 512], f32, name="psCS")
                for k, (row, lo, hi) in enumerate(CS_STRIPS):
                    nc.tensor.matmul(
                        cstile[:, 0:hi - lo],
                        lhsT=sel_sb[:, row, :], rhs=e_slice(lo, hi),
                        start=(k == 0), stop=(k == len(CS_STRIPS) - 1),
                    )
                csb = ep.tile([8, 512], f32, name="csb")
                nc.vector.tensor_copy(csb, cstile)
                nc.sync.dma_start(out=cs_dram[j], in_=csb)

            prev = None
            for j in range(NSTRIPE):
                tiles = emit_stripe(j)
                if prev is not None:
                    emit_cs(j - 1, prev)
                prev = tiles
                if j == NSTRIPE - 1:
                    nc.sync.dma_start(out=rs_dram[:, 0:3 * (NSTRIPE - 1)],
                                      in_=res_sb[:, 0:3 * (NSTRIPE - 1)])
            emit_cs(NSTRIPE - 1, prev)
            nc.sync.dma_start(out=rs_dram[:, 3 * (NSTRIPE - 1):],
                              in_=res_sb[:, 3 * (NSTRIPE - 1):])

    nc.compile()
    _cache["nc"] = nc
    return nc


def _make_static_inputs(h_i, h_j):
    h = np.concatenate([np.asarray(h_i), np.asarray(h_j)], axis=0).astype(np.float32)
    hT = np.ascontiguousarray(h.T)  # [256, 8192]
    hts = []
    for c in range(NCORES):
        htc = np.roll(hT, -B // 4 * c, axis=1)  # rotate by 1024*c
        pieces = [htc[:, s:e] for s, e in PIECES]
        cat = np.concatenate(pieces, axis=1)            # [256, HTW]
        hts.append(np.ascontiguousarray(
            cat.astype(ml_dtypes.float8_e4m3).reshape(2, 128, HTW)))
    p = np.arange(128)
    eye = np.zeros((128, 128), dtype=ml_dtypes.bfloat16)
    eye[p, p] = 1.0
    mskd = np.zeros((128, 128), dtype=ml_dtypes.bfloat16)
    mskd[p, p] = MASK_NEG
    sel = np.zeros((128, 8, 8), dtype=ml_dtypes.bfloat16)
    for i in range(8):
        sel[:, i, i] = 1.0
    return hts, eye, mskd, sel


def _assembly_indices():
    """Global-column index map for the colsum strips: [core, stripe, row, 512]."""
    idx = np.zeros((NCORES, NSTRIPE, 8, 512), dtype=np.int64)
    valid = np.zeros((NCORES, NSTRIPE, 8, 512), dtype=np.float64)
    for c in range(NCORES):
        for j in range(NSTRIPE):
            for row, lo, hi in CS_STRIPS:
                w = hi - lo
                loc = 128 * j + lo + np.arange(w)
                idx[c, j, row, :w] = (loc + 1024 * c) % N
                valid[c, j, row, :w] = 1.0
    return idx, valid


_IDX, _VALID = _assembly_indices()


def _axon_reset():
    try:
        import ctypes
        lib = ctypes.CDLL("/opt/axon/libaxon_pjrt.so")
        lib.axon_reset.restype = ctypes.c_int64
        return lib.axon_reset() == 0
    except Exception:
        return False


def _run(nc, hts, eye, mskd, sel):
    global LAST_RESULTS
    from concourse import bass_utils

    in_maps = [
        {"ht": hts[c], "eye": eye, "mskd": mskd, "sel": sel}
        for c in range(NCORES)
    ]
    try:
        results = bass_utils.run_bass_kernel_spmd(
            nc, in_maps, core_ids=list(range(NCORES)), trace=TRACE
        )
    except Exception:
        if not _axon_reset():
            raise
        results = bass_utils.run_bass_kernel_spmd(
            nc, in_maps, core_ids=list(range(NCORES)), trace=TRACE
        )
    LAST_RESULTS = results
    return results.results


def _host_fallback(h_i, h_j):
    """Exact float64 loss on the host (used only if the device result is
    numerically out of range for the fixed logsumexp shift)."""
    h = np.concatenate([np.asarray(h_i), np.asarray(h_j)], 0).astype(np.float64)
    sim = 2.0 * (h @ h.T)
    np.fill_diagonal(sim, -np.inf)
    m = sim.max(1)
    lse = m + np.log(np.exp(sim - m[:, None]).sum(1))
    pos = 2.0 * (h[:B] * h[B:]).sum(1)
    return np.float32((lse - np.concatenate([pos, pos])).mean())


def kernel(h_i, h_j):
    nc = _build()
    hts, eye, mskd, sel = _make_static_inputs(h_i, h_j)
    res = _run(nc, hts, eye, mskd, sel)

    S = np.zeros(N, dtype=np.float64)
    for c in range(NCORES):
        rs = res[c]["rs"].astype(np.float64)            # [128, 24]
        cs = res[c]["cs"].astype(np.float64)            # [8, 8, 512]
        # row sums: stripe j covers global rows 1024c + 128j + p
        rows = (1024 * c + (128 * np.arange(NSTRIPE))[:, None]
                + np.arange(128)[None, :])              # [8, 128]
        S[rows.ravel()] += rs.reshape(128, NSTRIPE, 3).sum(2).T.ravel()
        # column sums
        S += np.bincount(_IDX[c].ravel(),
                         weights=(cs * _VALID[c]).ravel(), minlength=N)

    if not (np.isfinite(S).all() and (S > 0.0).all()):
        return _host_fallback(h_i, h_j)

    lse = M_DEFAULT + np.log(S)
    h_i64 = np.asarray(h_i, dtype=np.float64)
    h_j64 = np.asarray(h_j, dtype=np.float64)
    pos = 2.0 * (h_i64 * h_j64).sum(1)
    loss = lse.mean() - pos.mean()
    return np.array(loss, dtype=np.float32)


if __name__ == "__main__":
    rng = np.random.default_rng(0)
    h_i = rng.standard_normal((B, D), dtype=np.float32)
    h_j = rng.standard_normal((B, D), dtype=np.float32)
    print("loss:", kernel(h_i, h_j))


# revision 12
# speedup vs baseline: 1.2830x; 1.0121x over previous
"""NT-Xent contrastive loss (forward) on 8 TRN2 NeuronCores via Bass/Tile.

Math: with h = concat(h_i, h_j) [N=8192, D=256], sim = (h @ h.T) / 0.5,
loss = mean_r( logsumexp_j(sim[r, j], j != r) - pos_r ), pos_r = 2 h_i[q].h_j[q].

sim is symmetric, so with a uniform logsumexp shift M the exp'd matrix
E = exp(sim - M) is symmetric too and each unordered block pair is computed
once: a circulant schedule where global row-block R (128 rows) covers the 33
column-blocks at cyclic offsets 0..32.  Offsets 1..31 contribute their row
sums to R's rows (free-axis accumulate) and their column sums (ones.T @ E on
the PE) to the transposed rows; offset 0 is the diagonal block (self-sim
masked by accumulating I.T @ (-1e9 diag) into PSUM); offset 32 is computed
by both endpoints, row-sums only.  Core c owns global row-blocks 8c..8c+7
with all columns pre-rotated by 1024c on the host, so one SPMD program
serves all 8 cores.

The sim blocks are produced by fp8(e4m3) DoubleRow matmuls (K=256 in one
pass, fp32 PSUM accumulate).  Per stripe of 4224 columns: two 1536-column
chunks go to the scalar engine (exact exp, bf16 out, fused row-sum) and
two 512 + one 128 column chunks go to the vector engine as a Schraudolph
u16 bit-trick exp whose bit pattern IS the bf16 value (round(A16*y+B16)),
so the same tile feeds both the row-reduce and the column-sum matmul.
The host assembles S_r from row/column partial sums, takes log in float64,
and computes the positive-pair term directly from the fp32 inputs.
"""

import numpy as np
import ml_dtypes

B = 4096
D = 256
N = 2 * B              # 8192 rows/cols of sim
NCORES = 8
NSTRIPE = 8            # row-blocks (stripes) per core
W = 4224               # stripe width: 33 blocks of 128 (cyclic offsets 0..32)
M_DEFAULT = 161.0      # logsumexp shift; safe while rowmax(2*h@h.T) in [M-70, M+79]
MASK_NEG = -1.0e9

# Schraudolph fast-exp in bf16 bit space: exp(y) ~= bitcast_bf16(round(A16*y+B16)).
A16 = 128.0 / np.log(2.0)
B16 = 16256.0 - 7.446   # offset calibrated to zero the exp-weighted mean error

# ht DMA pieces (local column ranges); every 512-aligned-128-offset chunk AP
# below fits inside exactly one piece (piece = min(x//1024, 4)).
PIECES = [(0, 1536), (1024, 2560), (2048, 3584), (3072, 4608), (4096, 5120)]
HTW = sum(e - s for s, e in PIECES)   # 7168

# per-stripe chunk layout (offsets relative to stripe start 128j)
#   A1 [0,1536) ACT   D1 [1536,2048) DVE   A2 [2048,3584) ACT
#   D2 [3584,4096) DVE   O3 [4096,4224) DVE (offset-32 block, both sides)
# The three DVE chunks share one u16 bits tile tD[0:512|512:1024|1024:1152]
# reduced by a single 1152-wide row-sum.
# colsum strips (psum row -> relative column range, all within one E tile);
# row 0 must be 512 wide (it carries start=True for the whole psum tile).
CS_STRIPS = [(0, 128, 640), (1, 640, 1152), (2, 1152, 1536), (3, 1536, 2048),
             (4, 3584, 4096), (5, 2048, 2560), (6, 2560, 3072), (7, 3072, 3584)]

TRACE = False
LAST_RESULTS = None

_cache = {}


def _build():
    if "nc" in _cache:
        return _cache["nc"]

    import concourse.tile as tile
    import concourse.mybir as mybir
    from concourse import bacc

    f32 = mybir.dt.float32
    bf16 = mybir.dt.bfloat16
    fp8 = mybir.dt.float8e4
    u16 = mybir.dt.uint16
    DR = mybir.MatmulPerfMode.DoubleRow

    nc = bacc.Bacc("TRN2", target_bir_lowering=False, num_devices=NCORES)
    ht_drams = [
        nc.dram_tensor(f"ht{i}", [128, 2, e - s], fp8, kind="ExternalInput").ap()
        for i, (s, e) in enumerate(PIECES)
    ]
    eye_dram = nc.dram_tensor("eye", [128, 128], bf16, kind="ExternalInput").ap()
    mskd_dram = nc.dram_tensor("mskd", [128, 128], bf16, kind="ExternalInput").ap()
    sel_dram = nc.dram_tensor("sel", [128, 8, 8], bf16, kind="ExternalInput").ap()
    rs_dram = nc.dram_tensor("rs", [128, 3 * NSTRIPE], f32, kind="ExternalOutput").ap()
    cs_dram = nc.dram_tensor("cs", [NSTRIPE, 8, 512], f32, kind="ExternalOutput").ap()

    TS1 = 2.0 * A16                  # psum holds X = a.a ; bits = A16*(2X - M) + B16
    TS2 = B16 - A16 * M_DEFAULT

    with tile.TileContext(nc) as tc:
        with (
            tc.tile_pool(name="hp", bufs=1) as hp,
            tc.tile_pool(name="small", bufs=1) as small,
            tc.tile_pool(name="ep", bufs=2) as ep,
            tc.tile_pool(name="psA", bufs=2, space="PSUM") as psA,
            tc.tile_pool(name="psB", bufs=1, space="PSUM") as psB,
            tc.tile_pool(name="psCS", bufs=1, space="PSUM") as psCS,
        ):
            # constants and ht pieces all on the sync HWDGE queue, most
            # critical first (mask consts gate stripe 0's first exp)
            mskd_sb = small.tile([128, 128], bf16)
            nc.sync.dma_start(out=mskd_sb, in_=mskd_dram)
            eye_sb = small.tile([128, 128], bf16)
            nc.sync.dma_start(out=eye_sb, in_=eye_dram)

            # per-partition ACT bias (-M) and exp-table warm tile
            biasm_sb = small.tile([128, 1], f32)
            nc.vector.memset(biasm_sb, -M_DEFAULT)
            warm_sb = small.tile([128, 1], f32)
            nc.vector.memset(warm_sb, 0.0)
            nc.scalar.activation(
                out=warm_sb, in_=warm_sb,
                func=mybir.ActivationFunctionType.Exp, bias=0.0, scale=0.0,
            )

            # warm the PE HAM clock gate with dummy matmuls while DMAs land
            wsrc = small.tile([128, 128], bf16)
            nc.vector.memset(wsrc, 0.0)
            wps = psA.tile([128, 1536], f32, name="psA")
            for w in range(16):
                nc.tensor.matmul(
                    wps[:, (w % 3) * 512:(w % 3) * 512 + 128],
                    lhsT=wsrc, rhs=wsrc, start=True, stop=True,
                )

            # ht pieces in consumption order; sel (first needed by the
            # stripe-0 colsums, ~6us in) rides between piece 0 and 1
            ht_tiles = []
            sel_sb = None
            for i, (c0, c1) in enumerate(PIECES):
                t = hp.tile([128, 2, c1 - c0], fp8, name=f"ht_{c0}")
                nc.sync.dma_start(out=t, in_=ht_drams[i])
                ht_tiles.append(t)
                if i == 0:
                    sel_sb = small.tile([128, 8, 8], bf16)
                    nc.sync.dma_start(out=sel_sb, in_=sel_dram)

            def rhs8(x, wdt=512):
                """[128, 2, wdt] fp8 slice of rotated ht at local column x."""
                p = min(x // 1024, 4)
                s, e = PIECES[p]
                assert s <= x and x + wdt <= e, (x, wdt)
                return ht_tiles[p][:, :, x - s:x - s + wdt]

            res_sb = small.tile([128, 3 * NSTRIPE], f32)

            def mm512(out, base_x, x, wdt=512):
                """Accumulate the K=256 fp8 product into one psum region via
                two K=128 matmuls (FWL hides their weight loads)."""
                for k in range(2):
                    nc.tensor.matmul(
                        out,
                        lhsT=ht_tiles[0][:, k, base_x:base_x + 128],
                        rhs=rhs8(x, wdt)[:, k, :],
                        start=(k == 0), stop=(k == 1),
                    )

            def emit_cs(j, tiles):
                """Column sums of stripe j (one stripe behind the produce)."""
                eA1, eA2, tD = tiles

                def e_slice(lo, hi):
                    if hi <= 1536:
                        return eA1[:, lo:hi]
                    if lo >= 1536 and hi <= 2048:
                        return tD.bitcast(bf16)[:, lo - 1536:hi - 1536]
                    if lo >= 2048 and hi <= 3584:
                        return eA2[:, lo - 2048:hi - 2048]
                    return tD.bitcast(bf16)[:, lo - 3584 + 512:hi - 3584 + 512]

                cstile = psCS.tile([8, 512], f32, name="psCS")
                for k, (row, lo, hi) in enumerate(CS_STRIPS):
                    nc.tensor.matmul(
                        cstile[:, 0:hi - lo],
                        lhsT=sel_sb[:, row, :], rhs=e_slice(lo, hi),
                        start=(k == 0), stop=(k == len(CS_STRIPS) - 1),
                    )
                csb = ep.tile([8, 512], f32, name="csb")
                nc.vector.tensor_copy(csb, cstile)
                nc.sync.dma_start(out=cs_dram[j], in_=csb)

            def emit_stripe(j, prev):
                """Produce + exp + row sums for stripe j, with stripe j-1's
                column sums slotted mid-stream; returns E tiles."""
                base = 128 * j
                rcol = 3 * j

                pA1 = psA.tile([128, 1536], f32, name="psA")
                for k in range(2):
                    for cs in range(3):
                        nc.tensor.matmul(
                            pA1[:, cs * 512:(cs + 1) * 512],
                            lhsT=ht_tiles[0][:, k, base:base + 128],
                            rhs=rhs8(base + cs * 512)[:, k, :],
                            start=(k == 0), stop=(k == 1) and (cs != 0),
                        )
                # mask self-similarity: += mskd[m, n] over cols [0,128)
                nc.tensor.matmul(
                    pA1[:, 0:128], lhsT=eye_sb, rhs=mskd_sb,
                    start=False, stop=True,
                )
                eA1 = ep.tile([128, 1536], bf16, name="eA1")
                nc.scalar.activation(
                    out=eA1, in_=pA1, func=mybir.ActivationFunctionType.Exp,
                    bias=biasm_sb, scale=2.0,
                    accum_out=res_sb[:, rcol:rcol + 1],
                )

                pA2 = psA.tile([128, 1536], f32, name="psA")
                for k in range(2):
                    for cs in range(3):
                        nc.tensor.matmul(
                            pA2[:, cs * 512:(cs + 1) * 512],
                            lhsT=ht_tiles[0][:, k, base:base + 128],
                            rhs=rhs8(base + 2048 + cs * 512)[:, k, :],
                            start=(k == 0), stop=(k == 1),
                        )
                eA2 = ep.tile([128, 1536], bf16, name="eA2")
                nc.scalar.activation(
                    out=eA2, in_=pA2, func=mybir.ActivationFunctionType.Exp,
                    bias=biasm_sb, scale=2.0,
                    accum_out=res_sb[:, rcol + 1:rcol + 2],
                )

                tD = ep.tile([128, 1152], u16, name="tD")
                pD1 = psB.tile([128, 512], f32, name="psB")
                mm512(pD1, base, base + 1536)
                nc.vector.tensor_scalar(
                    tD[:, 0:512], pD1, TS1, TS2,
                    mybir.AluOpType.mult, mybir.AluOpType.add)

                if prev is not None:
                    emit_cs(j - 1, prev)

                pD2 = psB.tile([128, 512], f32, name="psB")
                mm512(pD2, base, base + 3584)
                nc.vector.tensor_scalar(
                    tD[:, 512:1024], pD2, TS1, TS2,
                    mybir.AluOpType.mult, mybir.AluOpType.add)
                pO3 = psB.tile([128, 128], f32, name="psB")
                mm512(pO3, base, base + 4096, 128)
                nc.vector.tensor_scalar(
                    tD[:, 1024:1152], pO3, TS1, TS2,
                    mybir.AluOpType.mult, mybir.AluOpType.add)
                nc.vector.reduce_sum(
                    res_sb[:, rcol + 2:rcol + 3], tD.bitcast(bf16),
                    axis=mybir.AxisListType.X)
                return eA1, eA2, tD

            prev = None
            for j in range(NSTRIPE):
                prev = emit_stripe(j, prev)
                if j == NSTRIPE - 1:
                    nc.sync.dma_start(out=rs_dram[:, 0:3 * (NSTRIPE - 1)],
                                      in_=res_sb[:, 0:3 * (NSTRIPE - 1)])
            emit_cs(NSTRIPE - 1, prev)
            nc.sync.dma_start(out=rs_dram[:, 3 * (NSTRIPE - 1):],
                              in_=res_sb[:, 3 * (NSTRIPE - 1):])

    nc.compile()
    _cache["nc"] = nc
    return nc


def _make_static_inputs(h_i, h_j):
    h = np.concatenate([np.asarray(h_i), np.asarray(h_j)], axis=0).astype(np.float32)
    hT = np.ascontiguousarray(h.T)  # [256, 8192]
    hts = []
    for c in range(NCORES):
        htc = np.roll(hT, -B // 4 * c, axis=1).astype(ml_dtypes.float8_e4m3)
        pieces = {}
        for i, (s, e) in enumerate(PIECES):
            pieces[f"ht{i}"] = np.ascontiguousarray(
                htc[:, s:e].reshape(2, 128, e - s).transpose(1, 0, 2))
        hts.append(pieces)
    p = np.arange(128)
    eye = np.zeros((128, 128), dtype=ml_dtypes.bfloat16)
    eye[p, p] = 1.0
    mskd = np.zeros((128, 128), dtype=ml_dtypes.bfloat16)
    mskd[p, p] = MASK_NEG
    sel = np.zeros((128, 8, 8), dtype=ml_dtypes.bfloat16)
    for i in range(8):
        sel[:, i, i] = 1.0
    return hts, eye, mskd, sel


def _assembly_indices():
    """Global-column index map for the colsum strips: [core, stripe, row, 512]."""
    idx = np.zeros((NCORES, NSTRIPE, 8, 512), dtype=np.int64)
    valid = np.zeros((NCORES, NSTRIPE, 8, 512), dtype=np.float64)
    for c in range(NCORES):
        for j in range(NSTRIPE):
            for row, lo, hi in CS_STRIPS:
                w = hi - lo
                loc = 128 * j + lo + np.arange(w)
                idx[c, j, row, :w] = (loc + 1024 * c) % N
                valid[c, j, row, :w] = 1.0
    return idx, valid


_IDX, _VALID = _assembly_indices()


def _axon_reset():
    try:
        import ctypes
        lib = ctypes.CDLL("/opt/axon/libaxon_pjrt.so")
        lib.axon_reset.restype = ctypes.c_int64
        return lib.axon_reset() == 0
    except Exception:
        return False


def _run(nc, hts, eye, mskd, sel):
    global LAST_RESULTS
    from concourse import bass_utils

    in_maps = [
        {**hts[c], "eye": eye, "mskd": mskd, "sel": sel}
        for c in range(NCORES)
    ]
    try:
        results = bass_utils.run_bass_kernel_spmd(
            nc, in_maps, core_ids=list(range(NCORES)), trace=TRACE
        )
    except Exception:
        if not _axon_reset():
            raise
        results = bass_utils.run_bass_kernel_spmd(
            nc, in_maps, core_ids=list(range(NCORES)), trace=TRACE
        )
    LAST_RESULTS = results
    return results.results


def _host_fallback(h_i, h_j):
    """Exact float64 loss on the host (used only if the device result is
    numerically out of range for the fixed logsumexp shift)."""
    h = np.concatenate([np.asarray(h_i), np.asarray(h_j)], 0).astype(np.float64)
    sim = 2.0 * (h @ h.T)
    np.fill_diagonal(sim, -np.inf)
    m = sim.max(1)
    lse = m + np.log(np.exp(sim - m[:, None]).sum(1))
    pos = 2.0 * (h[:B] * h[B:]).sum(1)
    return np.float32((lse - np.concatenate([pos, pos])).mean())


def kernel(h_i, h_j):
    nc = _build()
    hts, eye, mskd, sel = _make_static_inputs(h_i, h_j)
    res = _run(nc, hts, eye, mskd, sel)

    S = np.zeros(N, dtype=np.float64)
    for c in range(NCORES):
        rs = res[c]["rs"].astype(np.float64)            # [128, 24]
        cs = res[c]["cs"].astype(np.float64)            # [8, 8, 512]
        # row sums: stripe j covers global rows 1024c + 128j + p
        rows = (1024 * c + (128 * np.arange(NSTRIPE))[:, None]
                + np.arange(128)[None, :])              # [8, 128]
        S[rows.ravel()] += rs.reshape(128, NSTRIPE, 3).sum(2).T.ravel()
        # column sums
        S += np.bincount(_IDX[c].ravel(),
                         weights=(cs * _VALID[c]).ravel(), minlength=N)

    if not (np.isfinite(S).all() and (S > 0.0).all()):
        return _host_fallback(h_i, h_j)

    lse = M_DEFAULT + np.log(S)
    h_i64 = np.asarray(h_i, dtype=np.float64)
    h_j64 = np.asarray(h_j, dtype=np.float64)
    pos = 2.0 * (h_i64 * h_j64).sum(1)
    loss = lse.mean() - pos.mean()
    return np.array(loss, dtype=np.float32)


if __name__ == "__main__":
    rng = np.random.default_rng(0)
    h_i = rng.standard_normal((B, D), dtype=np.float32)
    h_j = rng.standard_normal((B, D), dtype=np.float32)
    print("loss:", kernel(h_i, h_j))


# revision 16
# speedup vs baseline: 1.2909x; 1.0061x over previous
"""NT-Xent contrastive loss (forward) on 8 TRN2 NeuronCores via Bass/Tile.

Math: with h = concat(h_i, h_j) [N=8192, D=256], sim = (h @ h.T) / 0.5,
loss = mean_r( logsumexp_j(sim[r, j], j != r) - pos_r ), pos_r = 2 h_i[q].h_j[q].

sim is symmetric, so with a uniform logsumexp shift M the exp'd matrix
E = exp(sim - M) is symmetric too and each unordered block pair is computed
once: a circulant schedule where global row-block R (128 rows) covers the 33
column-blocks at cyclic offsets 0..32.  Offsets 1..31 contribute their row
sums to R's rows (free-axis accumulate) and their column sums (ones.T @ E on
the PE) to the transposed rows; offset 0 is the diagonal block (self-sim
masked by accumulating I.T @ (-1e9 diag) into PSUM); offset 32 is computed
by both endpoints, row-sums only.  Core c owns global row-blocks 8c..8c+7
with all columns pre-rotated by 1024c on the host, so one SPMD program
serves all 8 cores.

The sim blocks are produced by fp8(e4m3) DoubleRow matmuls (K=256 in one
pass, fp32 PSUM accumulate).  Per stripe of 4224 columns: two 1536-column
chunks go to the scalar engine (exact exp, bf16 out, fused row-sum) and
two 512 + one 128 column chunks go to the vector engine as a Schraudolph
u16 bit-trick exp whose bit pattern IS the bf16 value (round(A16*y+B16)),
so the same tile feeds both the row-reduce and the column-sum matmul.
The host assembles S_r from row/column partial sums, takes log in float64,
and computes the positive-pair term directly from the fp32 inputs.
"""

import numpy as np
import ml_dtypes

B = 4096
D = 256
N = 2 * B              # 8192 rows/cols of sim
NCORES = 8
NSTRIPE = 8            # row-blocks (stripes) per core
W = 4224               # stripe width: 33 blocks of 128 (cyclic offsets 0..32)
M_DEFAULT = 161.0      # logsumexp shift; safe while rowmax(2*h@h.T) in [M-70, M+79]
MASK_NEG = -1.0e9

# Schraudolph fast-exp in bf16 bit space: exp(y) ~= bitcast_bf16(round(A16*y+B16)).
A16 = 128.0 / np.log(2.0)
B16 = 16256.0 - 7.446   # offset calibrated to zero the exp-weighted mean error

# ht columns needed per core: [0, 5120), DMA'd in five 1024-col slices of
# one SBUF tile (sub-tile deps let each chunk start on its slice's arrival)
HTW = 5120

# per-stripe chunk layout (offsets relative to stripe start 128j)
#   A1 [0,1536) ACT   D1 [1536,2048) DVE   A2 [2048,3584) ACT
#   D2 [3584,4096) DVE   O3 [4096,4224) DVE (offset-32 block, both sides)
# The three DVE chunks share one u16 bits tile tD[0:512|512:1024|1024:1152]
# reduced by a single 1152-wide row-sum.
# colsum strips (psum row -> relative column range, all within one E tile);
# row 0 must be 512 wide (it carries start=True for the whole psum tile).
CS_STRIPS = [(0, 128, 640), (1, 640, 1152), (2, 1152, 1536), (3, 1536, 2048),
             (4, 3584, 4096), (5, 2048, 2560), (6, 2560, 3072), (7, 3072, 3584)]

TRACE = False
LAST_RESULTS = None

_cache = {}


def _build():
    if "nc" in _cache:
        return _cache["nc"]

    import concourse.tile as tile
    import concourse.mybir as mybir
    from concourse import bacc

    f32 = mybir.dt.float32
    bf16 = mybir.dt.bfloat16
    fp8 = mybir.dt.float8e4
    u16 = mybir.dt.uint16
    DR = mybir.MatmulPerfMode.DoubleRow

    nc = bacc.Bacc("TRN2", target_bir_lowering=False, num_devices=NCORES)
    ht_dram = nc.dram_tensor("ht", [128, 2, HTW], fp8, kind="ExternalInput").ap()
    eye_dram = nc.dram_tensor("eye", [128, 128], bf16, kind="ExternalInput").ap()
    mskd_dram = nc.dram_tensor("mskd", [128, 128], bf16, kind="ExternalInput").ap()
    sel_dram = nc.dram_tensor("sel", [128, 8, 8], bf16, kind="ExternalInput").ap()
    rs_dram = nc.dram_tensor("rs", [128, 3 * NSTRIPE], f32, kind="ExternalOutput").ap()
    cs_dram = nc.dram_tensor("cs", [NSTRIPE, 8, 512], f32, kind="ExternalOutput").ap()

    TS1 = 2.0 * A16                  # psum holds X = a.a ; bits = A16*(2X - M) + B16
    TS2 = B16 - A16 * M_DEFAULT

    with tile.TileContext(nc) as tc:
        with (
            tc.tile_pool(name="hp", bufs=1) as hp,
            tc.tile_pool(name="small", bufs=1) as small,
            tc.tile_pool(name="ep", bufs=2) as ep,
            tc.tile_pool(name="psA", bufs=2, space="PSUM") as psA,
            tc.tile_pool(name="psB", bufs=1, space="PSUM") as psB,
            tc.tile_pool(name="psCS", bufs=1, space="PSUM") as psCS,
        ):
            # constants and ht pieces all on the sync HWDGE queue, most
            # critical first (mask consts gate stripe 0's first exp)
            mskd_sb = small.tile([128, 128], bf16)
            nc.sync.dma_start(out=mskd_sb, in_=mskd_dram)
            eye_sb = small.tile([128, 128], bf16)
            nc.sync.dma_start(out=eye_sb, in_=eye_dram)

            # per-partition ACT bias (-M) and exp-table warm tile
            biasm_sb = small.tile([128, 1], f32)
            nc.vector.memset(biasm_sb, -M_DEFAULT)
            warm_sb = small.tile([128, 1], f32)
            nc.vector.memset(warm_sb, 0.0)
            nc.scalar.activation(
                out=warm_sb, in_=warm_sb,
                func=mybir.ActivationFunctionType.Exp, bias=0.0, scale=0.0,
            )

            # warm the PE HAM clock gate with dummy matmuls while DMAs land
            wsrc = small.tile([128, 128], bf16)
            nc.vector.memset(wsrc, 0.0)
            wps = psA.tile([128, 1536], f32, name="psA")
            for w in range(12):
                nc.tensor.matmul(
                    wps[:, (w % 3) * 512:(w % 3) * 512 + 128],
                    lhsT=wsrc, rhs=wsrc, start=True, stop=True,
                )

            # ht in five 1024-col slices, consumption order; sel (first
            # needed by the stripe-0 colsums) rides after slice 0
            ht_sb = hp.tile([128, 2, HTW], fp8)
            sel_sb = None
            for i in range(5):
                nc.sync.dma_start(out=ht_sb[:, :, 1024 * i:1024 * (i + 1)],
                                  in_=ht_dram[:, :, 1024 * i:1024 * (i + 1)])
                if i == 0:
                    sel_sb = small.tile([128, 8, 8], bf16)
                    nc.sync.dma_start(out=sel_sb, in_=sel_dram)

            def rhs8(x, wdt=512):
                """[128, 2, wdt] fp8 slice of rotated ht at local column x."""
                assert x + wdt <= HTW, (x, wdt)
                return ht_sb[:, :, x:x + wdt]

            res_sb = small.tile([128, 3 * NSTRIPE], f32)

            def mm512(out, base_x, x, wdt=512):
                """Accumulate the K=256 fp8 product into one psum region via
                two K=128 matmuls (FWL hides their weight loads)."""
                for k in range(2):
                    nc.tensor.matmul(
                        out,
                        lhsT=ht_sb[:, k, base_x:base_x + 128],
                        rhs=rhs8(x, wdt)[:, k, :],
                        start=(k == 0), stop=(k == 1),
                    )

            def emit_cs(j, tiles):
                """Column sums of stripe j (one stripe behind the produce)."""
                eA1, eA2, tD = tiles

                def e_slice(lo, hi):
                    if hi <= 1536:
                        return eA1[:, lo:hi]
                    if lo >= 1536 and hi <= 2048:
                        return tD.bitcast(bf16)[:, lo - 1536:hi - 1536]
                    if lo >= 2048 and hi <= 3584:
                        return eA2[:, lo - 2048:hi - 2048]
                    return tD.bitcast(bf16)[:, lo - 3584 + 512:hi - 3584 + 512]

                cstile = psCS.tile([8, 512], f32, name="psCS")
                for k, (row, lo, hi) in enumerate(CS_STRIPS):
                    nc.tensor.matmul(
                        cstile[:, 0:hi - lo],
                        lhsT=sel_sb[:, row, :], rhs=e_slice(lo, hi),
                        start=(k == 0), stop=(k == len(CS_STRIPS) - 1),
                    )
                csb = ep.tile([8, 512], f32, name="csb")
                nc.vector.tensor_copy(csb, cstile)
                nc.sync.dma_start(out=cs_dram[j], in_=csb)

            def emit_stripe(j, prev):
                """Produce + exp + row sums for stripe j, with stripe j-1's
                column sums slotted mid-stream; returns E tiles."""
                base = 128 * j
                rcol = 3 * j

                pA1 = psA.tile([128, 1536], f32, name="psA")
                for k in range(2):
                    for cs in range(3):
                        nc.tensor.matmul(
                            pA1[:, cs * 512:(cs + 1) * 512],
                            lhsT=ht_sb[:, k, base:base + 128],
                            rhs=rhs8(base + cs * 512)[:, k, :],
                            start=(k == 0), stop=(k == 1) and (cs != 0),
                        )
                # mask self-similarity: += mskd[m, n] over cols [0,128)
                nc.tensor.matmul(
                    pA1[:, 0:128], lhsT=eye_sb, rhs=mskd_sb,
                    start=False, stop=True,
                )
                eA1 = ep.tile([128, 1536], bf16, name="eA1")
                nc.scalar.activation(
                    out=eA1, in_=pA1, func=mybir.ActivationFunctionType.Exp,
                    bias=biasm_sb, scale=2.0,
                    accum_out=res_sb[:, rcol:rcol + 1],
                )

                tD = ep.tile([128, 1152], u16, name="tD")
                pD1 = psB.tile([128, 512], f32, name="psB")
                mm512(pD1, base, base + 1536)
                nc.vector.tensor_scalar(
                    tD[:, 0:512], pD1, TS1, TS2,
                    mybir.AluOpType.mult, mybir.AluOpType.add)

                pA2 = psA.tile([128, 1536], f32, name="psA")
                for k in range(2):
                    for cs in range(3):
                        nc.tensor.matmul(
                            pA2[:, cs * 512:(cs + 1) * 512],
                            lhsT=ht_sb[:, k, base:base + 128],
                            rhs=rhs8(base + 2048 + cs * 512)[:, k, :],
                            start=(k == 0), stop=(k == 1),
                        )
                eA2 = ep.tile([128, 1536], bf16, name="eA2")
                nc.scalar.activation(
                    out=eA2, in_=pA2, func=mybir.ActivationFunctionType.Exp,
                    bias=biasm_sb, scale=2.0,
                    accum_out=res_sb[:, rcol + 1:rcol + 2],
                )

                if prev is not None:
                    emit_cs(j - 1, prev)

                pD2 = psB.tile([128, 512], f32, name="psB")
                mm512(pD2, base, base + 3584)
                nc.vector.tensor_scalar(
                    tD[:, 512:1024], pD2, TS1, TS2,
                    mybir.AluOpType.mult, mybir.AluOpType.add)
                pO3 = psB.tile([128, 128], f32, name="psB")
                mm512(pO3, base, base + 4096, 128)
                nc.vector.tensor_scalar(
                    tD[:, 1024:1152], pO3, TS1, TS2,
                    mybir.AluOpType.mult, mybir.AluOpType.add)
                nc.vector.reduce_sum(
                    res_sb[:, rcol + 2:rcol + 3], tD.bitcast(bf16),
                    axis=mybir.AxisListType.X)
                return eA1, eA2, tD

            prev = None
            for j in range(NSTRIPE):
                prev = emit_stripe(j, prev)
                if j == NSTRIPE - 1:
                    nc.sync.dma_start(out=rs_dram[:, 0:3 * (NSTRIPE - 1)],
                                      in_=res_sb[:, 0:3 * (NSTRIPE - 1)])
            emit_cs(NSTRIPE - 1, prev)
            nc.sync.dma_start(out=rs_dram[:, 3 * (NSTRIPE - 1):],
                              in_=res_sb[:, 3 * (NSTRIPE - 1):])

    nc.compile()
    _cache["nc"] = nc
    return nc


def _make_static_inputs(h_i, h_j):
    h = np.concatenate([np.asarray(h_i), np.asarray(h_j)], axis=0).astype(np.float32)
    hT = np.ascontiguousarray(h.T)  # [256, 8192]
    hts = []
    for c in range(NCORES):
        htc = np.roll(hT, -B // 4 * c, axis=1).astype(ml_dtypes.float8_e4m3)
        hts.append({"ht": np.ascontiguousarray(
            htc[:, :HTW].reshape(2, 128, HTW).transpose(1, 0, 2))})
    p = np.arange(128)
    eye = np.zeros((128, 128), dtype=ml_dtypes.bfloat16)
    eye[p, p] = 1.0
    mskd = np.zeros((128, 128), dtype=ml_dtypes.bfloat16)
    mskd[p, p] = MASK_NEG
    sel = np.zeros((128, 8, 8), dtype=ml_dtypes.bfloat16)
    for i in range(8):
        sel[:, i, i] = 1.0
    return hts, eye, mskd, sel


def _assembly_indices():
    """Global-column index map for the colsum strips: [core, stripe, row, 512]."""
    idx = np.zeros((NCORES, NSTRIPE, 8, 512), dtype=np.int64)
    valid = np.zeros((NCORES, NSTRIPE, 8, 512), dtype=np.float64)
    for c in range(NCORES):
        for j in range(NSTRIPE):
            for row, lo, hi in CS_STRIPS:
                w = hi - lo
                loc = 128 * j + lo + np.arange(w)
                idx[c, j, row, :w] = (loc + 1024 * c) % N
                valid[c, j, row, :w] = 1.0
    return idx, valid


_IDX, _VALID = _assembly_indices()


def _axon_reset():
    try:
        import ctypes
        lib = ctypes.CDLL("/opt/axon/libaxon_pjrt.so")
        lib.axon_reset.restype = ctypes.c_int64
        return lib.axon_reset() == 0
    except Exception:
        return False


def _run(nc, hts, eye, mskd, sel):
    global LAST_RESULTS
    from concourse import bass_utils

    in_maps = [
        {**hts[c], "eye": eye, "mskd": mskd, "sel": sel}
        for c in range(NCORES)
    ]
    try:
        results = bass_utils.run_bass_kernel_spmd(
            nc, in_maps, core_ids=list(range(NCORES)), trace=TRACE
        )
    except Exception:
        if not _axon_reset():
            raise
        results = bass_utils.run_bass_kernel_spmd(
            nc, in_maps, core_ids=list(range(NCORES)), trace=TRACE
        )
    LAST_RESULTS = results
    return results.results


def _host_fallback(h_i, h_j):
    """Exact float64 loss on the host (used only if the device result is
    numerically out of range for the fixed logsumexp shift)."""
    h = np.concatenate([np.asarray(h_i), np.asarray(h_j)], 0).astype(np.float64)
    sim = 2.0 * (h @ h.T)
    np.fill_diagonal(sim, -np.inf)
    m = sim.max(1)
    lse = m + np.log(np.exp(sim - m[:, None]).sum(1))
    pos = 2.0 * (h[:B] * h[B:]).sum(1)
    return np.float32((lse - np.concatenate([pos, pos])).mean())


def kernel(h_i, h_j):
    nc = _build()
    hts, eye, mskd, sel = _make_static_inputs(h_i, h_j)
    res = _run(nc, hts, eye, mskd, sel)

    S = np.zeros(N, dtype=np.float64)
    for c in range(NCORES):
        rs = res[c]["rs"].astype(np.float64)            # [128, 24]
        cs = res[c]["cs"].astype(np.float64)            # [8, 8, 512]
        # row sums: stripe j covers global rows 1024c + 128j + p
        rows = (1024 * c + (128 * np.arange(NSTRIPE))[:, None]
                + np.arange(128)[None, :])              # [8, 128]
        S[rows.ravel()] += rs.reshape(128, NSTRIPE, 3).sum(2).T.ravel()
        # column sums
        S += np.bincount(_IDX[c].ravel(),
                         weights=(cs * _VALID[c]).ravel(), minlength=N)

    if not (np.isfinite(S).all() and (S > 0.0).all()):
        return _host_fallback(h_i, h_j)

    lse = M_DEFAULT + np.log(S)
    h_i64 = np.asarray(h_i, dtype=np.float64)
    h_j64 = np.asarray(h_j, dtype=np.float64)
    pos = 2.0 * (h_i64 * h_j64).sum(1)
    loss = lse.mean() - pos.mean()
    return np.array(loss, dtype=np.float32)


if __name__ == "__main__":
    rng = np.random.default_rng(0)
    h_i = rng.standard_normal((B, D), dtype=np.float32)
    h_j = rng.standard_normal((B, D), dtype=np.float32)
    print("loss:", kernel(h_i, h_j))


# revision 17
# speedup vs baseline: 1.3617x; 1.0549x over previous
"""NT-Xent contrastive loss (forward) on 8 TRN2 NeuronCores via Bass/Tile.

Math: with h = concat(h_i, h_j) [N=8192, D=256], sim = (h @ h.T) / 0.5,
loss = mean_r( logsumexp_j(sim[r, j], j != r) - pos_r ), pos_r = 2 h_i[q].h_j[q].

sim is symmetric, so with a uniform logsumexp shift M the exp'd matrix
E = exp(sim - M) is symmetric too and each unordered block pair is computed
once: a circulant schedule where global row-block R (128 rows) covers the 33
column-blocks at cyclic offsets 0..32.  Offsets 1..31 contribute their row
sums to R's rows (free-axis accumulate) and their column sums (ones.T @ E on
the PE) to the transposed rows; offset 0 is the diagonal block (self-sim
masked by accumulating I.T @ (-1e9 diag) into PSUM); offset 32 is computed
by both endpoints, row-sums only.  Core c owns global row-blocks 8c..8c+7
with all columns pre-rotated by 1024c on the host, so one SPMD program
serves all 8 cores.

The sim blocks are produced by fp8(e4m3) DoubleRow matmuls (K=256 in one
pass, fp32 PSUM accumulate).  Per stripe of 4224 columns: two 1536-column
chunks go to the scalar engine (exact exp, bf16 out, fused row-sum) and
two 512 + one 128 column chunks go to the vector engine as a Schraudolph
u16 bit-trick exp whose bit pattern IS the bf16 value (round(A16*y+B16)),
so the same tile feeds both the row-reduce and the column-sum matmul.
The host assembles S_r from row/column partial sums, takes log in float64,
and computes the positive-pair term directly from the fp32 inputs.
"""

import numpy as np
import ml_dtypes

B = 4096
D = 256
N = 2 * B              # 8192 rows/cols of sim
NCORES = 8
NSTRIPE = 8            # row-blocks (stripes) per core
W = 4224               # stripe width: 33 blocks of 128 (cyclic offsets 0..32)
M_DEFAULT = 161.0      # logsumexp shift; safe while rowmax(2*h@h.T) in [M-70, M+79]
MASK_NEG = -1.0e9

# Schraudolph fast-exp in bf16 bit space: exp(y) ~= bitcast_bf16(round(A16*y+B16)).
A16 = 128.0 / np.log(2.0)
B16 = 16256.0 - 7.446   # offset calibrated to zero the exp-weighted mean error

# ht columns needed per core: [0, 5120), DMA'd in five 1024-col slices of
# one SBUF tile (sub-tile deps let each chunk start on its slice's arrival)
HTW = 5120

# per-stripe chunk layout (offsets relative to stripe start 128j)
#   A1 [0,1536) ACT   D1 [1536,2048) DVE   A2 [2048,3584) ACT
#   D2 [3584,4096) DVE   O3 [4096,4224) DVE (offset-32 block, both sides)
# The three DVE chunks share one u16 bits tile tD[0:512|512:1024|1024:1152]
# reduced by a single 1152-wide row-sum.
# colsum strips (psum row -> relative column range, all within one E tile);
# row 0 must be 512 wide (it carries start=True for the whole psum tile).
CS_STRIPS = [(0, 128, 640), (1, 640, 1152), (2, 1152, 1536), (3, 1536, 2048),
             (4, 3584, 4096), (5, 2048, 2560), (6, 2560, 3072), (7, 3072, 3584)]

TRACE = False
LAST_RESULTS = None

_cache = {}


def _build():
    if "nc" in _cache:
        return _cache["nc"]

    import concourse.tile as tile
    import concourse.mybir as mybir
    from concourse import bacc

    f32 = mybir.dt.float32
    bf16 = mybir.dt.bfloat16
    fp8 = mybir.dt.float8e4
    u16 = mybir.dt.uint16
    DR = mybir.MatmulPerfMode.DoubleRow

    nc = bacc.Bacc("TRN2", target_bir_lowering=False, num_devices=NCORES)
    ht_dram = nc.dram_tensor("ht", [128, 2, HTW], fp8, kind="ExternalInput").ap()
    eye_dram = nc.dram_tensor("eye", [128, 128], bf16, kind="ExternalInput").ap()
    mskd_dram = nc.dram_tensor("mskd", [128, 128], bf16, kind="ExternalInput").ap()
    sel_dram = nc.dram_tensor("sel", [128, 8, 8], bf16, kind="ExternalInput").ap()
    rs_dram = nc.dram_tensor("rs", [128, 3 * NSTRIPE], f32, kind="ExternalOutput").ap()
    cs_dram = nc.dram_tensor("cs", [NSTRIPE, 8, 512], f32, kind="ExternalOutput").ap()

    TS1 = 2.0 * A16                  # psum holds X = a.a ; bits = A16*(2X - M) + B16
    TS2 = B16 - A16 * M_DEFAULT

    with tile.TileContext(nc) as tc:
        with (
            tc.tile_pool(name="hp", bufs=1) as hp,
            tc.tile_pool(name="small", bufs=1) as small,
            tc.tile_pool(name="ep", bufs=2) as ep,
            tc.tile_pool(name="psA", bufs=2, space="PSUM") as psA,
            tc.tile_pool(name="psB", bufs=1, space="PSUM") as psB,
            tc.tile_pool(name="psCS", bufs=1, space="PSUM") as psCS,
        ):

            # per-partition ACT bias (-M) and exp-table warm tile
            biasm_sb = small.tile([128, 1], f32)
            nc.vector.memset(biasm_sb, -M_DEFAULT)
            warm_sb = small.tile([128, 1], f32)
            nc.vector.memset(warm_sb, 0.0)
            nc.scalar.activation(
                out=warm_sb, in_=warm_sb,
                func=mybir.ActivationFunctionType.Exp, bias=0.0, scale=0.0,
            )

            # warm the PE HAM clock gate with dummy matmuls while DMAs land
            wsrc = small.tile([128, 128], bf16)
            nc.vector.memset(wsrc, 0.0)
            wps = psA.tile([128, 1536], f32, name="psA")
            for w in range(12):
                nc.tensor.matmul(
                    wps[:, (w % 3) * 512:(w % 3) * 512 + 128],
                    lhsT=wsrc, rhs=wsrc, start=True, stop=True,
                )

            # ht lands in three 2048-col slice DMAs (2KB runs per partition,
            # sub-tile deps let chunks start per slice); the small constants
            # ride the same sync queue between slices, criticality-ordered
            ht_sb = hp.tile([128, 2, HTW], fp8)

            def ht_slice_dma(c0, c1):
                nc.sync.dma_start(out=ht_sb[:, :, c0:c1],
                                  in_=ht_dram[:, :, c0:c1])

            ht_slice_dma(0, 2048)
            mskd_sb = small.tile([128, 128], bf16)
            nc.sync.dma_start(out=mskd_sb, in_=mskd_dram)
            eye_sb = small.tile([128, 128], bf16)
            nc.sync.dma_start(out=eye_sb, in_=eye_dram)
            ht_slice_dma(2048, 4096)
            sel_sb = small.tile([128, 8, 8], bf16)
            nc.sync.dma_start(out=sel_sb, in_=sel_dram)
            ht_slice_dma(4096, 5120)

            def rhs8(x, wdt=512):
                """[128, 2, wdt] fp8 slice of rotated ht at local column x."""
                assert x + wdt <= HTW, (x, wdt)
                return ht_sb[:, :, x:x + wdt]

            res_sb = small.tile([128, 3 * NSTRIPE], f32)

            def mm512(out, base_x, x, wdt=512):
                """Accumulate the K=256 fp8 product into one psum region via
                two K=128 matmuls (FWL hides their weight loads)."""
                for k in range(2):
                    nc.tensor.matmul(
                        out,
                        lhsT=ht_sb[:, k, base_x:base_x + 128],
                        rhs=rhs8(x, wdt)[:, k, :],
                        start=(k == 0), stop=(k == 1),
                    )

            def emit_cs(j, tiles):
                """Column sums of stripe j (one stripe behind the produce)."""
                eA1, eA2, tD = tiles

                def e_slice(lo, hi):
                    if hi <= 1536:
                        return eA1[:, lo:hi]
                    if lo >= 1536 and hi <= 2048:
                        return tD.bitcast(bf16)[:, lo - 1536:hi - 1536]
                    if lo >= 2048 and hi <= 3584:
                        return eA2[:, lo - 2048:hi - 2048]
                    return tD.bitcast(bf16)[:, lo - 3584 + 512:hi - 3584 + 512]

                cstile = psCS.tile([8, 512], f32, name="psCS")
                for k, (row, lo, hi) in enumerate(CS_STRIPS):
                    nc.tensor.matmul(
                        cstile[:, 0:hi - lo],
                        lhsT=sel_sb[:, row, :], rhs=e_slice(lo, hi),
                        start=(k == 0), stop=(k == len(CS_STRIPS) - 1),
                    )
                csb = ep.tile([8, 512], f32, name="csb")
                if j == NSTRIPE - 1:
                    nc.scalar.copy(csb, cstile)
                else:
                    nc.vector.tensor_copy(csb, cstile)
                nc.sync.dma_start(out=cs_dram[j], in_=csb)

            def emit_stripe(j, prev):
                """Produce + exp + row sums for stripe j, with stripe j-1's
                column sums slotted mid-stream; returns E tiles."""
                base = 128 * j
                rcol = 3 * j

                pA1 = psA.tile([128, 1536], f32, name="psA")
                for k in range(2):
                    for cs in range(3):
                        nc.tensor.matmul(
                            pA1[:, cs * 512:(cs + 1) * 512],
                            lhsT=ht_sb[:, k, base:base + 128],
                            rhs=rhs8(base + cs * 512)[:, k, :],
                            start=(k == 0), stop=(k == 1) and (cs != 0),
                        )
                # mask self-similarity: += mskd[m, n] over cols [0,128)
                nc.tensor.matmul(
                    pA1[:, 0:128], lhsT=eye_sb, rhs=mskd_sb,
                    start=False, stop=True,
                )
                eA1 = ep.tile([128, 1536], bf16, name="eA1")
                nc.scalar.activation(
                    out=eA1, in_=pA1, func=mybir.ActivationFunctionType.Exp,
                    bias=biasm_sb, scale=2.0,
                    accum_out=res_sb[:, rcol:rcol + 1],
                )

                tD = ep.tile([128, 1152], u16, name="tD")
                pD1 = psB.tile([128, 512], f32, name="psB")
                mm512(pD1, base, base + 1536)
                nc.vector.tensor_scalar(
                    tD[:, 0:512], pD1, TS1, TS2,
                    mybir.AluOpType.mult, mybir.AluOpType.add)

                pA2 = psA.tile([128, 1536], f32, name="psA")
                for k in range(2):
                    for cs in range(3):
                        nc.tensor.matmul(
                            pA2[:, cs * 512:(cs + 1) * 512],
                            lhsT=ht_sb[:, k, base:base + 128],
                            rhs=rhs8(base + 2048 + cs * 512)[:, k, :],
                            start=(k == 0), stop=(k == 1),
                        )
                eA2 = ep.tile([128, 1536], bf16, name="eA2")
                nc.scalar.activation(
                    out=eA2, in_=pA2, func=mybir.ActivationFunctionType.Exp,
                    bias=biasm_sb, scale=2.0,
                    accum_out=res_sb[:, rcol + 1:rcol + 2],
                )

                if prev is not None:
                    emit_cs(j - 1, prev)

                pD2 = psB.tile([128, 512], f32, name="psB")
                mm512(pD2, base, base + 3584)
                nc.vector.tensor_scalar(
                    tD[:, 512:1024], pD2, TS1, TS2,
                    mybir.AluOpType.mult, mybir.AluOpType.add)
                pO3 = psB.tile([128, 128], f32, name="psB")
                mm512(pO3, base, base + 4096, 128)
                nc.vector.tensor_scalar(
                    tD[:, 1024:1152], pO3, TS1, TS2,
                    mybir.AluOpType.mult, mybir.AluOpType.add)
                nc.vector.reduce_sum(
                    res_sb[:, rcol + 2:rcol + 3], tD.bitcast(bf16),
                    axis=mybir.AxisListType.X)
                return eA1, eA2, tD

            prev = None
            for j in range(NSTRIPE):
                prev = emit_stripe(j, prev)
                if j == NSTRIPE - 1:
                    nc.sync.dma_start(out=rs_dram[:, 0:3 * (NSTRIPE - 1)],
                                      in_=res_sb[:, 0:3 * (NSTRIPE - 1)])
            emit_cs(NSTRIPE - 1, prev)
            nc.sync.dma_start(out=rs_dram[:, 3 * (NSTRIPE - 1):],
                              in_=res_sb[:, 3 * (NSTRIPE - 1):])

    nc.compile()
    _cache["nc"] = nc
    return nc


def _make_static_inputs(h_i, h_j):
    h = np.concatenate([np.asarray(h_i), np.asarray(h_j)], axis=0).astype(np.float32)
    hT = np.ascontiguousarray(h.T)  # [256, 8192]
    hts = []
    for c in range(NCORES):
        htc = np.roll(hT, -B // 4 * c, axis=1).astype(ml_dtypes.float8_e4m3)
        hts.append({"ht": np.ascontiguousarray(
            htc[:, :HTW].reshape(2, 128, HTW).transpose(1, 0, 2))})
    p = np.arange(128)
    eye = np.zeros((128, 128), dtype=ml_dtypes.bfloat16)
    eye[p, p] = 1.0
    mskd = np.zeros((128, 128), dtype=ml_dtypes.bfloat16)
    mskd[p, p] = MASK_NEG
    sel = np.zeros((128, 8, 8), dtype=ml_dtypes.bfloat16)
    for i in range(8):
        sel[:, i, i] = 1.0
    return hts, eye, mskd, sel


def _assembly_indices():
    """Global-column index map for the colsum strips: [core, stripe, row, 512]."""
    idx = np.zeros((NCORES, NSTRIPE, 8, 512), dtype=np.int64)
    valid = np.zeros((NCORES, NSTRIPE, 8, 512), dtype=np.float64)
    for c in range(NCORES):
        for j in range(NSTRIPE):
            for row, lo, hi in CS_STRIPS:
                w = hi - lo
                loc = 128 * j + lo + np.arange(w)
                idx[c, j, row, :w] = (loc + 1024 * c) % N
                valid[c, j, row, :w] = 1.0
    return idx, valid


_IDX, _VALID = _assembly_indices()


def _axon_reset():
    try:
        import ctypes
        lib = ctypes.CDLL("/opt/axon/libaxon_pjrt.so")
        lib.axon_reset.restype = ctypes.c_int64
        return lib.axon_reset() == 0
    except Exception:
        return False


def _run(nc, hts, eye, mskd, sel):
    global LAST_RESULTS
    from concourse import bass_utils

    in_maps = [
        {**hts[c], "eye": eye, "mskd": mskd, "sel": sel}
        for c in range(NCORES)
    ]
    try:
        results = bass_utils.run_bass_kernel_spmd(
            nc, in_maps, core_ids=list(range(NCORES)), trace=TRACE
        )
    except Exception:
        if not _axon_reset():
            raise
        results = bass_utils.run_bass_kernel_spmd(
            nc, in_maps, core_ids=list(range(NCORES)), trace=TRACE
        )
    LAST_RESULTS = results
    return results.results


def _host_fallback(h_i, h_j):
    """Exact float64 loss on the host (used only if the device result is
    numerically out of range for the fixed logsumexp shift)."""
    h = np.concatenate([np.asarray(h_i), np.asarray(h_j)], 0).astype(np.float64)
    sim = 2.0 * (h @ h.T)
    np.fill_diagonal(sim, -np.inf)
    m = sim.max(1)
    lse = m + np.log(np.exp(sim - m[:, None]).sum(1))
    pos = 2.0 * (h[:B] * h[B:]).sum(1)
    return np.float32((lse - np.concatenate([pos, pos])).mean())


def kernel(h_i, h_j):
    nc = _build()
    hts, eye, mskd, sel = _make_static_inputs(h_i, h_j)
    res = _run(nc, hts, eye, mskd, sel)

    S = np.zeros(N, dtype=np.float64)
    for c in range(NCORES):
        rs = res[c]["rs"].astype(np.float64)            # [128, 24]
        cs = res[c]["cs"].astype(np.float64)            # [8, 8, 512]
        # row sums: stripe j covers global rows 1024c + 128j + p
        rows = (1024 * c + (128 * np.arange(NSTRIPE))[:, None]
                + np.arange(128)[None, :])              # [8, 128]
        S[rows.ravel()] += rs.reshape(128, NSTRIPE, 3).sum(2).T.ravel()
        # column sums
        S += np.bincount(_IDX[c].ravel(),
                         weights=(cs * _VALID[c]).ravel(), minlength=N)

    if not (np.isfinite(S).all() and (S > 0.0).all()):
        return _host_fallback(h_i, h_j)

    lse = M_DEFAULT + np.log(S)
    h_i64 = np.asarray(h_i, dtype=np.float64)
    h_j64 = np.asarray(h_j, dtype=np.float64)
    pos = 2.0 * (h_i64 * h_j64).sum(1)
    loss = lse.mean() - pos.mean()
    return np.array(loss, dtype=np.float32)


if __name__ == "__main__":
    rng = np.random.default_rng(0)
    h_i = rng.standard_normal((B, D), dtype=np.float32)
    h_j = rng.standard_normal((B, D), dtype=np.float32)
    print("loss:", kernel(h_i, h_j))
